# revision 26
# baseline (speedup 1.0000x reference)
"""Causal single-head attention (B=4, S=2048, E=1024, fp32) on 8 TRN2 NeuronCores.

Sharding: data-parallel over batch (4) x 2-way causal-balanced query split at
256-row granularity.  The sequence stays in causal order on every core; core
(b, par) owns 256-row query chunks {0,3,4,7} (par=0) or {1,2,5,6} (par=1),
shipped separately as xq (x^T restricted to the owned query columns).  The
device program is identical on all 8 cores (SPMD): program q-block j attends
key units [0 : 2*(j+1)*256); the owned chunks are assigned to blocks sorted
by causal need (need(c) = c+1 256-key-units), which by construction satisfies
need in {sched-1, sched}, so only the last two key units of each block ever
carry a mask (triangular diagonal / all-dead / all-live), applied from
per-core mask data via an identity matmul into the score PSUM.

Algebra: both weight applications are hoisted off the attention inner loop.
scores = (x@M) @ x^T with M = W_Q^T W_K precomputed on the host (kills the
K projection), and out = (P @ x) @ W_V^T (kills the V projection): the device
accumulates O1^j = P_j @ x in PSUM, normalizes by the softmax row-sum during
the PSUM->SBUF copy, transposes O1 on the PE, and applies W_V^T once per
128-row query tile.

All matmul operands are bf16 (PE rate is identical to f32r at free >= 256,
but DMA, SBUF, and copy traffic halve; max rel err vs the f32 reference is
~5e-3, well inside the 2e-2 gate).  Scores and O1 accumulate in f32 PSUM.
DMAs are few and consolidated (the HWDGE ring is FIFO, so issue order =
transfer order; dep-chains cost ~2.2us per link in DGE restarts).  The
per-block tails (normalize, O1 transpose, W_V projection, store) are
software-pipelined behind the next block's first score group.
"""

import numpy as np

B, S, E = 4, 2048, 1024
P = 128          # partitions
C = 512
Q = 256          # query block granularity
NEG = -1e9
NCORES = 8
SCALE = 1.0 / np.sqrt(np.float32(E))

_CHUNKS = {0: (0, 3, 4, 7), 1: (1, 2, 5, 6)}   # owned 256-chunks per par

_CACHE = {}


def _install_drain_patch():
    """walrus in this env fits only 1 sync wait per CTRL_NO instruction; split
    the TileContext end-of-kernel drain waits across trailing SP nops."""
    import concourse.mybir as mybir
    import concourse.tile as tile
    from concourse.vector_clock import ScopedClock

    if getattr(tile.TileContext, "_drain_split_installed", False):
        return

    def _split_drain_and_barrier(self, tick_clock, wait_clock):
        drain_inst = self.nc.sync.drain()
        wait_clock.add_sem_waits(
            drain_inst.ins, ScopedClock({None: tick_clock.global_clock})
        )
        si = drain_inst.ins.sync_info
        waits = list(si.on_wait) if si and si.on_wait else []
        if len(waits) > 1:
            si.on_wait = waits[:1]
            rest = waits[1:]
            while rest:
                chunk, rest = rest[:1], rest[1:]
                nop = self.nc.sync.nop(nofuse=True, hint="drain_wait_split")
                nsi = nop.ins.sync_info
                if nsi is None:
                    nop.ins.sync_info = mybir.SyncInfo(on_wait=chunk, on_update=[])
                else:
                    nsi.on_wait = list(nsi.on_wait) + chunk

        self.nc.all_engine_barrier()
        assert self.sems is not None
        popped = self.nc._tile_sem_poison_stack.pop()
        assert popped is self._sem_poison
        self.nc.clear_and_free_semaphores(list(self.sems.allocated().values()))
        self.nc.all_engine_barrier()

    tile.TileContext._drain_and_barrier = _split_drain_and_barrier
    tile.TileContext._drain_split_installed = True


def _split_excess_waits(nc, limit=1):
    """walrus here fits only `limit` sync waits per instruction; move excess
    waits of every instruction onto injected same-engine NoOps placed directly
    before it (program order on the engine preserves the semantics)."""
    import copy

    import concourse.mybir as mybir

    template = None
    for f in nc.m.functions:
        for bb in f.blocks:
            for inst in bb.instructions:
                if type(inst).__name__ == "InstNoOp":
                    template = inst
                    break
            if template is not None:
                break
        if template is not None:
            break
    assert template is not None, "no InstNoOp template found"

    n = 0
    for f in nc.m.functions:
        for bb in f.blocks:
            new = []
            for inst in bb.instructions:
                si = inst.sync_info
                waits = list(si.on_wait) if si and si.on_wait else []
                if len(waits) > limit:
                    si.on_wait = waits[-limit:]
                    excess = waits[:-limit]
                    while excess:
                        chunk, excess = excess[:limit], excess[limit:]
                        nop = copy.copy(template)
                        nop.name = f"I-wsplit-{n}"
                        n += 1
                        nop.engine = inst.engine
                        nop.sync_info = mybir.SyncInfo(on_wait=chunk, on_update=[])
                        import bass_rust

                        nop.set_nosync_dependencies(
                            bass_rust.InstructionNameOrderedSet()
                        )
                        nop.set_sync_dependencies(
                            bass_rust.InstructionNameOrderedSet()
                        )
                        new.append(nop)
                new.append(inst)
            bb.instructions[:] = new
    return n


def _build_program():
    """One SPMD program; per-core behaviour differs only through input data."""
    import concourse.bass as bass
    import concourse.mybir as mybir
    import concourse.tile as tile
    from concourse.masks import make_identity

    _install_drain_patch()

    f32 = mybir.dt.float32
    f32r = mybir.dt.float32r
    bf16 = mybir.dt.bfloat16
    Act = mybir.ActivationFunctionType

    nc = bass.Bass(dynamic_dma_scratch_size=128)
    xT = nc.declare_dram_parameter("xT", [E, S], bf16, isOutput=False)
    xq = nc.declare_dram_parameter("xq", [E, 2 * C], bf16, isOutput=False)
    xk = nc.declare_dram_parameter("xk", [S, E], bf16, isOutput=False)
    wm = nc.declare_dram_parameter("wm", [E, E], bf16, isOutput=False)
    wv = nc.declare_dram_parameter("wv", [E, E], bf16, isOutput=False)
    masks = nc.declare_dram_parameter("masks", [P, 16 * Q], bf16, isOutput=False)
    out = nc.declare_dram_parameter("out", [2 * C, E], f32, isOutput=True)

    xT_r = xT.rearrange("(et p) s -> p et s", p=P)      # [128, 8, 2048]
    xq_r = xq.rearrange("(et p) q -> p et q", p=P)      # [128, 8, 1024]
    xk_r = xk.rearrange("(kt p) e -> p kt e", p=P)      # [128, 16, 1024]
    wm_r = wm.rearrange("(et p) d -> p et d", p=P)      # [128, 8, 1024]
    wv_r = wv.rearrange("(et p) d -> p et d", p=P)

    ET = E // P   # 8 contraction tiles
    DT = E // P   # 8 head-dim tiles
    KTiles = S // P  # 16 key tiles

    with tile.TileContext(nc) as tc:
        from contextlib import ExitStack

        with ExitStack() as ctx:
            big = ctx.enter_context(tc.tile_pool(name="big", bufs=1))
            mpool = ctx.enter_context(tc.tile_pool(name="mask", bufs=1))
            kvp = ctx.enter_context(tc.tile_pool(name="kv", bufs=1))
            ident = mpool.tile([P, P], f32)
            make_identity(nc, ident)
            ident_r = mpool.tile([P, P], f32r)
            nc.vector.tensor_copy(ident_r[:], ident[:])
            ident_bf = mpool.tile([P, P], bf16)
            nc.vector.tensor_copy(ident_bf[:], ident[:])
            masks_sb = mpool.tile([P, 16 * Q], bf16)
            zbias = mpool.tile([P, 1], f32)
            nc.vector.memset(zbias[:], 0.0)
            xk_sb = kvp.tile([P, KTiles, E], bf16, tag="xk")
            wv_sb = kvp.tile([P, ET, E], bf16, tag="wvf")
            xt_sb = big.tile([P, ET, S], bf16, tag="xt")    # x^T [e, s]
            gt_sb0 = big.tile([P, DT, C], bf16, tag="gt0")  # G^T [e, q] j0|j1
            gt_sb1 = big.tile([P, DT, C], bf16, tag="gt1")  # G^T [e, q] j2|j3

            # ---- G^T = M^T xq^T for the core's 1024 owned query columns ----
            with ExitStack() as pctx:
                wmp = pctx.enter_context(tc.tile_pool(name="wm", bufs=1))
                xqp = pctx.enter_context(tc.tile_pool(name="xq", bufs=1))
                gpsum = pctx.enter_context(
                    tc.tile_pool(name="gpsum", bufs=1, space="PSUM")
                )

                wm_sb = wmp.tile([P, ET, E], bf16, tag="wm")
                xq_sb = xqp.tile([P, ET, 2 * C], bf16, tag="xq")
                # Consolidated DMAs, no dep chains: the HWDGE ring is FIFO,
                # so issue order = transfer order at full bandwidth.  Chained
                # DMAs pay ~2.2us of DGE-restart latency per link.
                nc.sync.dma_start(wm_sb[:, 0, 0:C], wm_r[:, 0, 0:C])
                nc.sync.dma_start(xq_sb[:, 0, 0:C], xq_r[:, 0, 0:C])
                nc.sync.dma_start(wm_sb[:, 0, C:E], wm_r[:, 0, C:E])
                for et in range(1, ET):
                    nc.sync.dma_start(wm_sb[:, et, :], wm_r[:, et, :])
                    nc.sync.dma_start(
                        xq_sb[:, et, 0:C], xq_r[:, et, 0:C]
                    )
                nc.sync.dma_start(
                    xq_sb[:, :, C : 2 * C], xq_r[:, :, C : 2 * C]
                )
                # non-critical inputs, in first-use order
                nc.sync.dma_start(xt_sb[:, :, 0:C], xT_r[:, :, 0:C])
                nc.sync.dma_start(masks_sb[:], masks[:])
                nc.sync.dma_start(xk_sb[:, 0:4, :], xk_r[:, 0:4, :])
                nc.sync.dma_start(wv_sb[:], wv_r[:])
                nc.sync.dma_start(xt_sb[:, :, C : 2 * C], xT_r[:, :, C : 2 * C])
                nc.sync.dma_start(xk_sb[:, 4:8, :], xk_r[:, 4:8, :])
                nc.sync.dma_start(
                    xt_sb[:, :, 2 * C : 4 * C], xT_r[:, :, 2 * C : 4 * C]
                )
                nc.sync.dma_start(xk_sb[:, 8:16, :], xk_r[:, 8:16, :])

                for qb in range(2):
                    gps = [
                        gpsum.tile([P, C], f32, tag=f"gp{dt}", name=f"gp{qb}_{dt}")
                        for dt in range(DT)
                    ]
                    gdst = gt_sb0 if qb == 0 else gt_sb1
                    for dt in range(DT):
                        for et in range(ET):
                            nc.tensor.matmul(
                                gps[dt][:],
                                wm_sb[:, et, bass.ts(dt, P)],
                                xq_sb[:, et, bass.ts(qb, C)],
                                start=(et == 0),
                                stop=(et == ET - 1),
                            )
                        if dt == DT - 1:
                            nc.vector.tensor_copy(
                                gdst[:, dt, 0:C // 2], gps[dt][:, 0:C // 2]
                            )
                            nc.scalar.activation(
                                gdst[:, dt, C // 2 : C],
                                gps[dt][:, C // 2 : C],
                                Act.Copy,
                                scale=1.0,
                            )
                        elif dt % 2 == 0:
                            nc.vector.tensor_copy(gdst[:, dt, :], gps[dt][:])
                        else:
                            nc.scalar.activation(
                                gdst[:, dt, :], gps[dt][:], Act.Copy, scale=1.0
                            )

            # ---- attention: per q-block j (256 rows, r in {0,1}), key units
            # ku in [0, 2*(j+1)): scores -> exp -> P^T -> O1 += P^T-tile @ x,
            # normalize O1 by recip(rowsum) in the PSUM->SBUF copy, transpose
            # O1 on the PE, then out = O1 @ W_V^T.  Tails are pipelined into
            # the next block's first score group. ----
            with ExitStack() as actx:
                ppool = actx.enter_context(tc.tile_pool(name="p", bufs=4))
                ptpool = actx.enter_context(tc.tile_pool(name="pt", bufs=6))
                o1pool = actx.enter_context(tc.tile_pool(name="o1", bufs=2))
                o1tp = actx.enter_context(tc.tile_pool(name="o1t", bufs=2))
                obuf = actx.enter_context(tc.tile_pool(name="ob", bufs=4))
                stat = actx.enter_context(tc.tile_pool(name="stat", bufs=8))
                spsum = actx.enter_context(
                    tc.tile_pool(name="spsum", bufs=2, space="PSUM")
                )
                o1psum = actx.enter_context(
                    tc.tile_pool(name="o1ps", bufs=2, space="PSUM")
                )
                ptpsum = actx.enter_context(
                    tc.tile_pool(name="ptpsum", bufs=2, space="PSUM")
                )

                def emit_tail_norm(state):
                    j, r, nk, o_lo, o_hi, sums = state
                    stot = stat.tile([P, 1], f32, tag="stot", name="stot")
                    nc.vector.reduce_sum(
                        stot[:], sums[:, 0:nk], axis=mybir.AxisListType.X
                    )
                    recip = stat.tile([P, 1], f32, tag="recip", name="recip")
                    nc.vector.reciprocal(recip[:], stot[:])
                    # normalized O1 (softmax denominator applied here, so the
                    # final projection needs no epilogue scale), in quarter
                    # tiles split across Act and DVE so the first transpose
                    # input is ready fast
                    o1nq = []
                    for qq in range(4):
                        src_ps = o_lo if qq < 2 else o_hi
                        piece = o1pool.tile(
                            [P, Q], bf16, tag=f"o1nq{qq}", name="o1nq"
                        )
                        if qq % 2 == 0:
                            nc.scalar.activation(
                                piece[:],
                                src_ps[:, bass.ts(qq % 2, Q)],
                                Act.Copy,
                                scale=recip[:],
                            )
                        else:
                            nc.vector.tensor_scalar_mul(
                                piece[:], src_ps[:, bass.ts(qq % 2, Q)], recip[:]
                            )
                        o1nq.append(piece)
                    return (j, r, o1nq)

                def emit_tail(state, fine=False):
                    j, r, o1nq = state
                    o1ts = []
                    for et in range(ET):
                        piece = o1nq[et // 2]
                        tps = ptpsum.tile([P, P], bf16, tag="ptps", name="tps")
                        nc.tensor.transpose(
                            tps[:], piece[:, bass.ts(et % 2, P)], ident_bf[:]
                        )
                        o1t = o1tp.tile([P, P], bf16, tag=f"o1t{et}", name="o1t")
                        nc.vector.tensor_copy(o1t[:], tps[:])
                        o1ts.append(o1t)
                    nq = 4 if fine else 2
                    w = E // nq
                    for piece in range(nq):
                        fp = spsum.tile([P, w], f32, tag="s", name="fp")
                        for et in range(ET):
                            nc.tensor.matmul(
                                fp[:],
                                o1ts[et][:],
                                wv_sb[:, et, bass.ds(piece * w, w)],
                                start=(et == 0),
                                stop=(et == ET - 1),
                            )
                        obh = obuf.tile(
                            [P, w], f32, tag=f"ob{piece % 2}", name="obh"
                        )
                        if piece % 2 == 0:
                            nc.scalar.activation(
                                obh[:], fp[:], Act.Copy, scale=1.0
                            )
                        else:
                            nc.vector.tensor_copy(obh[:], fp[:])
                        nc.sync.dma_start(
                            out[bass.ds((j * 2 + r) * P, P), bass.ds(piece * w, w)],
                            obh[:],
                        )

                def emit_block(j, r, pending):
                    if pending is not None:
                        pending = emit_tail_norm(pending)
                    nk = 2 * (j + 1)
                    gsrc = gt_sb0 if j < 2 else gt_sb1
                    qcol0 = (j % 2) * Q + r * P
                    o_lo = o1psum.tile([P, C], f32, tag="olo", name="olo")
                    o_hi = o1psum.tile([P, C], f32, tag="ohi", name="ohi")
                    sums = stat.tile([P, 8], f32, tag="sums", name="sums")
                    for ku in range(nk):
                        s_t = spsum.tile([P, Q], f32, tag="s", name="s_t")
                        masked = ku >= nk - 2
                        for dt in range(DT):
                            nc.tensor.matmul(
                                s_t[:],
                                gsrc[:, dt, bass.ds(qcol0, P)],
                                xt_sb[:, dt, bass.ts(ku, Q)],
                                start=(dt == 0),
                                stop=(dt == DT - 1 and not masked),
                            )
                        if masked:
                            slot = j * 4 + (ku - (nk - 2)) * 2 + r
                            nc.tensor.matmul(
                                s_t[:],
                                ident_bf[:],
                                masks_sb[:, bass.ts(slot, Q)],
                                start=False,
                                stop=True,
                            )
                        p_t = ppool.tile([P, Q], bf16, tag="p", name="p_t")
                        nc.scalar.activation(
                            p_t[:],
                            s_t[:],
                            Act.Exp,
                            bias=zbias[:],
                            scale=float(SCALE),
                            accum_out=sums[:, ku : ku + 1],
                        )
                        if ku == 1 and pending is not None:
                            emit_tail(pending)
                            pending = None
                        pts = []
                        for ks in range(Q // P):
                            pt_ps = ptpsum.tile(
                                [P, P], bf16, tag="ptps", name="pt_ps"
                            )
                            nc.tensor.transpose(
                                pt_ps[:], p_t[:, bass.ts(ks, P)], ident_bf[:]
                            )
                            pt_sb = ptpool.tile(
                                [P, P], bf16, tag="ptsb", name="pt_sb"
                            )
                            nc.vector.tensor_copy(pt_sb[:], pt_ps[:])
                            pts.append(pt_sb)
                        for ks in range(Q // P):
                            kt_idx = ku * 2 + ks
                            first = ku == 0 and ks == 0
                            last = ku == nk - 1 and ks == Q // P - 1
                            nc.tensor.matmul(
                                o_lo[:],
                                pts[ks][:],
                                xk_sb[:, kt_idx, 0:C],
                                start=first,
                                stop=last,
                            )
                            nc.tensor.matmul(
                                o_hi[:],
                                pts[ks][:],
                                xk_sb[:, kt_idx, C:E],
                                start=first,
                                stop=last,
                            )
                    return (j, r, nk, o_lo, o_hi, sums)

                pending = None
                for j in range(4):
                    for r in range(2):
                        pending = emit_block(j, r, pending)
                emit_tail(emit_tail_norm(pending), fine=True)
    _split_excess_waits(nc)
    return nc


def _build_masks(par):
    """16 mask slots [P, 256] (bf16 on the wire): slot j*4 + kui*2 + r covers
    key unit ku = 2*(j+1)-2+kui for q-rows of owned chunk j, row tile r."""
    chunks = _CHUNKS[par]
    m = np.zeros((P, 16, Q), np.float32)
    for j in range(4):
        nk = 2 * (j + 1)
        c = chunks[j]
        need = c + 1
        for kui in range(2):
            ku = nk - 2 + kui
            for r in range(2):
                slot = j * 4 + kui * 2 + r
                if ku < need - 1:
                    continue  # fully live, zero mask
                if ku == need - 1:
                    qpos = c * Q + r * P + np.arange(P)[:, None]
                    kpos = ku * Q + np.arange(Q)[None, :]
                    m[:, slot] = np.where(kpos <= qpos, 0.0, np.float32(NEG))
                else:
                    m[:, slot] = NEG
    return np.ascontiguousarray(m.reshape(P, 16 * Q))


def _host_inputs(x, W_Q, W_K, W_V):
    """Per-core input maps (host-side prep: chunk selection + W_Q^T W_K)."""
    import ml_dtypes

    bf = ml_dtypes.bfloat16
    x = np.ascontiguousarray(np.asarray(x, dtype=np.float32))
    wm = np.ascontiguousarray(
        (np.asarray(W_Q, np.float64).T @ np.asarray(W_K, np.float64)).astype(bf)
    )
    wvT = np.ascontiguousarray(np.asarray(W_V, np.float32).T.astype(bf))
    in_maps = []
    for c in range(NCORES):
        b, par = c // 2, c % 2
        xb = x[b]
        xq_rows = np.concatenate(
            [xb[ch * Q : (ch + 1) * Q] for ch in _CHUNKS[par]]
        )
        in_maps.append(
            {
                "xT": np.ascontiguousarray(xb.T.astype(bf)),
                "xq": np.ascontiguousarray(xq_rows.T.astype(bf)),
                "xk": np.ascontiguousarray(xb.astype(bf)),
                "wm": wm,
                "wv": wvT,
                "masks": _build_masks(par).astype(bf),
            }
        )
    return in_maps


def kernel(x, W_Q, W_K, W_V):
    from concourse.bass_utils import run_bass_kernel_spmd

    if "nc" not in _CACHE:
        _CACHE["nc"] = _build_program()
    nc = _CACHE["nc"]

    in_maps = _host_inputs(x, W_Q, W_K, W_V)
    res = run_bass_kernel_spmd(nc, in_maps, list(range(NCORES)))

    out = np.empty((B, S, E), np.float32)
    for c in range(NCORES):
        b, par = c // 2, c % 2
        o = res.results[c]["out"]  # [1024, 1024]
        for j, ch in enumerate(_CHUNKS[par]):
            out[b, ch * Q : (ch + 1) * Q] = o[j * Q : (j + 1) * Q]
    return out


# revision 32
# speedup vs baseline: 1.0163x; 1.0163x over previous
"""Causal single-head attention (B=4, S=2048, E=1024, fp32) on 8 TRN2 NeuronCores.

Sharding: data-parallel over batch (4) x 2-way causal-balanced query split at
256-row granularity.  The sequence stays in causal order on every core; core
(b, par) owns 256-row query chunks {0,3,4,7} (par=0) or {1,2,5,6} (par=1),
shipped separately as xq (x^T restricted to the owned query columns).  The
device program is identical on all 8 cores (SPMD): program q-block j attends
key units [0 : 2*(j+1)*256); the owned chunks are assigned to blocks sorted
by causal need (need(c) = c+1 256-key-units), which by construction satisfies
need in {sched-1, sched}, so only the last two key units of each block ever
carry a mask (triangular diagonal / all-dead / all-live), applied from
per-core mask data via an identity matmul into the score PSUM.

Algebra: both weight applications are hoisted off the attention inner loop.
scores = (x@M) @ x^T with M = W_Q^T W_K precomputed on the host (kills the
K projection), and out = (P @ x) @ W_V^T (kills the V projection): the device
accumulates O1^j = P_j @ x in PSUM, normalizes by the softmax row-sum during
the PSUM->SBUF copy, transposes O1 on the PE, and applies W_V^T once per
128-row query tile.

All matmul operands are bf16 (PE rate is identical to f32r at free >= 256,
but DMA, SBUF, and copy traffic halve; max rel err vs the f32 reference is
~5e-3, well inside the 2e-2 gate).  Scores and O1 accumulate in f32 PSUM.
DMAs are few and consolidated (the HWDGE ring is FIFO, so issue order =
transfer order; dep-chains cost ~2.2us per link in DGE restarts).  The
per-block tails (normalize, O1 transpose, W_V projection, store) are
software-pipelined behind the next block's first score group.
"""

import numpy as np

B, S, E = 4, 2048, 1024
P = 128          # partitions
C = 512
Q = 256          # query block granularity
NEG = -1e9
NCORES = 8
SCALE = 1.0 / np.sqrt(np.float32(E))

_CHUNKS = {0: (0, 3, 4, 7), 1: (1, 2, 5, 6)}   # owned 256-chunks per par

_CACHE = {}


def _install_drain_patch():
    """walrus in this env fits only 1 sync wait per CTRL_NO instruction; split
    the TileContext end-of-kernel drain waits across trailing SP nops."""
    import concourse.mybir as mybir
    import concourse.tile as tile
    from concourse.vector_clock import ScopedClock

    if getattr(tile.TileContext, "_drain_split_installed", False):
        return

    def _split_drain_and_barrier(self, tick_clock, wait_clock):
        drain_inst = self.nc.sync.drain()
        wait_clock.add_sem_waits(
            drain_inst.ins, ScopedClock({None: tick_clock.global_clock})
        )
        si = drain_inst.ins.sync_info
        waits = list(si.on_wait) if si and si.on_wait else []
        if len(waits) > 1:
            si.on_wait = waits[:1]
            rest = waits[1:]
            while rest:
                chunk, rest = rest[:1], rest[1:]
                nop = self.nc.sync.nop(nofuse=True, hint="drain_wait_split")
                nsi = nop.ins.sync_info
                if nsi is None:
                    nop.ins.sync_info = mybir.SyncInfo(on_wait=chunk, on_update=[])
                else:
                    nsi.on_wait = list(nsi.on_wait) + chunk

        self.nc.all_engine_barrier()
        assert self.sems is not None
        popped = self.nc._tile_sem_poison_stack.pop()
        assert popped is self._sem_poison
        self.nc.clear_and_free_semaphores(list(self.sems.allocated().values()))
        self.nc.all_engine_barrier()

    tile.TileContext._drain_and_barrier = _split_drain_and_barrier
    tile.TileContext._drain_split_installed = True


def _split_excess_waits(nc, limit=1):
    """walrus here fits only `limit` sync waits per instruction; move excess
    waits of every instruction onto injected same-engine NoOps placed directly
    before it (program order on the engine preserves the semantics)."""
    import copy

    import concourse.mybir as mybir

    template = None
    for f in nc.m.functions:
        for bb in f.blocks:
            for inst in bb.instructions:
                if type(inst).__name__ == "InstNoOp":
                    template = inst
                    break
            if template is not None:
                break
        if template is not None:
            break
    assert template is not None, "no InstNoOp template found"

    n = 0
    for f in nc.m.functions:
        for bb in f.blocks:
            new = []
            for inst in bb.instructions:
                si = inst.sync_info
                waits = list(si.on_wait) if si and si.on_wait else []
                if len(waits) > limit:
                    si.on_wait = waits[-limit:]
                    excess = waits[:-limit]
                    while excess:
                        chunk, excess = excess[:limit], excess[limit:]
                        nop = copy.copy(template)
                        nop.name = f"I-wsplit-{n}"
                        n += 1
                        nop.engine = inst.engine
                        nop.sync_info = mybir.SyncInfo(on_wait=chunk, on_update=[])
                        import bass_rust

                        nop.set_nosync_dependencies(
                            bass_rust.InstructionNameOrderedSet()
                        )
                        nop.set_sync_dependencies(
                            bass_rust.InstructionNameOrderedSet()
                        )
                        new.append(nop)
                new.append(inst)
            bb.instructions[:] = new
    return n


def _build_program():
    """One SPMD program; per-core behaviour differs only through input data."""
    import concourse.bass as bass
    import concourse.mybir as mybir
    import concourse.tile as tile
    from concourse.masks import make_identity

    _install_drain_patch()

    f32 = mybir.dt.float32
    f32r = mybir.dt.float32r
    bf16 = mybir.dt.bfloat16
    Act = mybir.ActivationFunctionType

    nc = bass.Bass(dynamic_dma_scratch_size=128)
    xT = nc.declare_dram_parameter("xT", [E, S], bf16, isOutput=False)
    xq = nc.declare_dram_parameter("xq", [E, 2 * C], bf16, isOutput=False)
    xk = nc.declare_dram_parameter("xk", [S, E], bf16, isOutput=False)
    wm = nc.declare_dram_parameter("wm", [E, E], bf16, isOutput=False)
    wv = nc.declare_dram_parameter("wv", [E, E], bf16, isOutput=False)
    masks = nc.declare_dram_parameter("masks", [P, 16 * Q], bf16, isOutput=False)
    out = nc.declare_dram_parameter("out", [2 * C, E], f32, isOutput=True)

    xT_r = xT.rearrange("(et p) s -> p et s", p=P)      # [128, 8, 2048]
    xq_r = xq.rearrange("(et p) q -> p et q", p=P)      # [128, 8, 1024]
    xk_r = xk.rearrange("(kt p) e -> p kt e", p=P)      # [128, 16, 1024]
    wm_r = wm.rearrange("(et p) d -> p et d", p=P)      # [128, 8, 1024]
    wv_r = wv.rearrange("(et p) d -> p et d", p=P)

    ET = E // P   # 8 contraction tiles
    DT = E // P   # 8 head-dim tiles
    KTiles = S // P  # 16 key tiles

    with tile.TileContext(nc) as tc:
        from contextlib import ExitStack

        with ExitStack() as ctx:
            big = ctx.enter_context(tc.tile_pool(name="big", bufs=1))
            mpool = ctx.enter_context(tc.tile_pool(name="mask", bufs=1))
            kvp = ctx.enter_context(tc.tile_pool(name="kv", bufs=1))
            ident = mpool.tile([P, P], f32)
            make_identity(nc, ident)
            ident_r = mpool.tile([P, P], f32r)
            nc.vector.tensor_copy(ident_r[:], ident[:])
            ident_bf = mpool.tile([P, P], bf16)
            nc.vector.tensor_copy(ident_bf[:], ident[:])
            masks_sb = mpool.tile([P, 16 * Q], bf16)
            zbias = mpool.tile([P, 1], f32)
            nc.vector.memset(zbias[:], 0.0)
            xk_sb = kvp.tile([P, KTiles, E], bf16, tag="xk")
            wv_sb = kvp.tile([P, ET, E], bf16, tag="wvf")
            xt_sb = big.tile([P, ET, S], bf16, tag="xt")    # x^T [e, s]
            gt_sb0 = big.tile([P, DT, C], bf16, tag="gt0")  # G^T [e, q] j0|j1
            gt_sb1 = big.tile([P, DT, C], bf16, tag="gt1")  # G^T [e, q] j2|j3

            # ---- G^T = M^T xq^T for the core's 1024 owned query columns ----
            with ExitStack() as pctx:
                wmp = pctx.enter_context(tc.tile_pool(name="wm", bufs=1))
                xqp = pctx.enter_context(tc.tile_pool(name="xq", bufs=1))
                gpsum = pctx.enter_context(
                    tc.tile_pool(name="gpsum", bufs=1, space="PSUM")
                )

                wm_sb = wmp.tile([P, ET, E], bf16, tag="wm")
                xq_sb = xqp.tile([P, ET, 2 * C], bf16, tag="xq")
                # Consolidated DMAs, no dep chains: the HWDGE ring is FIFO,
                # so issue order = transfer order at full bandwidth.  Chained
                # DMAs pay ~2.2us of DGE-restart latency per link.
                nc.sync.dma_start(wm_sb[:, 0, 0:C], wm_r[:, 0, 0:C])
                nc.sync.dma_start(xq_sb[:, 0, 0:C], xq_r[:, 0, 0:C])
                nc.sync.dma_start(wm_sb[:, 0, C:E], wm_r[:, 0, C:E])
                for et in range(1, ET):
                    nc.sync.dma_start(wm_sb[:, et, :], wm_r[:, et, :])
                    nc.sync.dma_start(
                        xq_sb[:, et, 0:C], xq_r[:, et, 0:C]
                    )
                nc.sync.dma_start(
                    xq_sb[:, :, C : 2 * C], xq_r[:, :, C : 2 * C]
                )
                # non-critical inputs, in first-use order
                nc.sync.dma_start(xt_sb[:, :, 0:C], xT_r[:, :, 0:C])
                nc.sync.dma_start(masks_sb[:], masks[:])
                nc.sync.dma_start(xk_sb[:, 0:4, :], xk_r[:, 0:4, :])
                nc.sync.dma_start(wv_sb[:], wv_r[:])
                nc.sync.dma_start(xt_sb[:, :, C : 2 * C], xT_r[:, :, C : 2 * C])
                nc.sync.dma_start(xk_sb[:, 4:8, :], xk_r[:, 4:8, :])
                nc.sync.dma_start(
                    xt_sb[:, :, 2 * C : 4 * C], xT_r[:, :, 2 * C : 4 * C]
                )
                nc.sync.dma_start(xk_sb[:, 8:16, :], xk_r[:, 8:16, :])

                for qb in range(2):
                    gps = [
                        gpsum.tile([P, C], f32, tag=f"gp{dt}", name=f"gp{qb}_{dt}")
                        for dt in range(DT)
                    ]
                    gdst = gt_sb0 if qb == 0 else gt_sb1
                    for dt in range(DT):
                        for et in range(ET):
                            nc.tensor.matmul(
                                gps[dt][:],
                                wm_sb[:, et, bass.ts(dt, P)],
                                xq_sb[:, et, bass.ts(qb, C)],
                                start=(et == 0),
                                stop=(et == ET - 1),
                            )
                        if dt == DT - 1:
                            nc.vector.tensor_copy(
                                gdst[:, dt, 0:C // 2], gps[dt][:, 0:C // 2]
                            )
                            nc.scalar.activation(
                                gdst[:, dt, C // 2 : C],
                                gps[dt][:, C // 2 : C],
                                Act.Copy,
                                scale=1.0,
                            )
                        elif dt % 2 == 0:
                            nc.vector.tensor_copy(gdst[:, dt, :], gps[dt][:])
                        else:
                            nc.scalar.activation(
                                gdst[:, dt, :], gps[dt][:], Act.Copy, scale=1.0
                            )

            # ---- attention: per q-block j (256 rows, r in {0,1}), key units
            # ku in [0, 2*(j+1)): scores -> exp -> P^T -> O1 += P^T-tile @ x,
            # normalize O1 by recip(rowsum) in the PSUM->SBUF copy, transpose
            # O1 on the PE, then out = O1 @ W_V^T.  Tails are pipelined into
            # the next block's first score group. ----
            with ExitStack() as actx:
                ppool = actx.enter_context(tc.tile_pool(name="p", bufs=4))
                ptpool = actx.enter_context(tc.tile_pool(name="pt", bufs=6))
                o1pool = actx.enter_context(tc.tile_pool(name="o1", bufs=2))
                o1tp = actx.enter_context(tc.tile_pool(name="o1t", bufs=2))
                obuf = actx.enter_context(tc.tile_pool(name="ob", bufs=4))
                stat = actx.enter_context(tc.tile_pool(name="stat", bufs=8))
                spsum = actx.enter_context(
                    tc.tile_pool(name="spsum", bufs=2, space="PSUM")
                )
                o1psum = actx.enter_context(
                    tc.tile_pool(name="o1ps", bufs=2, space="PSUM")
                )
                ptpsum = actx.enter_context(
                    tc.tile_pool(name="ptpsum", bufs=2, space="PSUM")
                )

                def emit_tail_norm(state):
                    j, r, nu, o_lo, o_hi, sums = state
                    stot = stat.tile([P, 1], f32, tag="stot", name="stot")
                    nc.vector.reduce_sum(
                        stot[:], sums[:, 0:nu], axis=mybir.AxisListType.X
                    )
                    recip = stat.tile([P, 1], f32, tag="recip", name="recip")
                    nc.vector.reciprocal(recip[:], stot[:])
                    # normalized O1 (softmax denominator applied here, so the
                    # final projection needs no epilogue scale), in quarter
                    # tiles split across Act and DVE so the first transpose
                    # input is ready fast
                    o1nq = []
                    for qq in range(4):
                        src_ps = o_lo if qq < 2 else o_hi
                        piece = o1pool.tile(
                            [P, Q], bf16, tag=f"o1nq{qq}", name="o1nq"
                        )
                        if qq % 2 == 0:
                            nc.scalar.activation(
                                piece[:],
                                src_ps[:, bass.ts(qq % 2, Q)],
                                Act.Copy,
                                scale=recip[:],
                            )
                        else:
                            nc.vector.tensor_scalar_mul(
                                piece[:], src_ps[:, bass.ts(qq % 2, Q)], recip[:]
                            )
                        o1nq.append(piece)
                    return (j, r, o1nq)

                def emit_tail(state, fine=False):
                    j, r, o1nq = state
                    o1ts = []
                    for et in range(ET):
                        piece = o1nq[et // 2]
                        tps = ptpsum.tile([P, P], bf16, tag="ptps", name="tps")
                        nc.tensor.transpose(
                            tps[:], piece[:, bass.ts(et % 2, P)], ident_bf[:]
                        )
                        o1t = o1tp.tile([P, P], bf16, tag=f"o1t{et}", name="o1t")
                        nc.vector.tensor_copy(o1t[:], tps[:])
                        o1ts.append(o1t)
                    nq = 4 if fine else 2
                    w = E // nq
                    for piece in range(nq):
                        fp = spsum.tile([P, w], f32, tag="s", name="fp")
                        for et in range(ET):
                            nc.tensor.matmul(
                                fp[:],
                                o1ts[et][:],
                                wv_sb[:, et, bass.ds(piece * w, w)],
                                start=(et == 0),
                                stop=(et == ET - 1),
                            )
                        obh = obuf.tile(
                            [P, w], f32, tag=f"ob{piece % 2}", name="obh"
                        )
                        if piece % 2 == 0:
                            nc.scalar.activation(
                                obh[:], fp[:], Act.Copy, scale=1.0
                            )
                        else:
                            nc.vector.tensor_copy(obh[:], fp[:])
                        nc.sync.dma_start(
                            out[bass.ds((j * 2 + r) * P, P), bass.ds(piece * w, w)],
                            obh[:],
                        )

                def emit_block(j, r, pending):
                    if pending is not None:
                        pending = emit_tail_norm(pending)
                    nk = 2 * (j + 1)
                    gsrc = gt_sb0 if j < 2 else gt_sb1
                    qcol0 = (j % 2) * Q + r * P
                    o_lo = o1psum.tile([P, C], f32, tag="olo", name="olo")
                    o_hi = o1psum.tile([P, C], f32, tag="ohi", name="ohi")
                    sums = stat.tile([P, 8], f32, tag="sums", name="sums")
                    # units: fused 512-wide pairs over the unmasked prefix,
                    # then the two masked 256-wide key units
                    units = [(2 * pi, 2) for pi in range((nk - 2) // 2)]
                    units += [(nk - 2, 1), (nk - 1, 1)]
                    for ui, (ku0, width) in enumerate(units):
                        kw = width * Q
                        s_t = spsum.tile([P, kw], f32, tag="s", name="s_t")
                        masked = ku0 >= nk - 2
                        for dt in range(DT):
                            nc.tensor.matmul(
                                s_t[:],
                                gsrc[:, dt, bass.ds(qcol0, P)],
                                xt_sb[:, dt, bass.ds(ku0 * Q, kw)],
                                start=(dt == 0),
                                stop=(dt == DT - 1 and not masked),
                            )
                        if masked:
                            slot = j * 4 + (ku0 - (nk - 2)) * 2 + r
                            nc.tensor.matmul(
                                s_t[:],
                                ident_bf[:],
                                masks_sb[:, bass.ts(slot, Q)],
                                start=False,
                                stop=True,
                            )
                        p_t = ppool.tile([P, kw], bf16, tag="p", name="p_t")
                        nc.scalar.activation(
                            p_t[:],
                            s_t[:],
                            Act.Exp,
                            bias=zbias[:],
                            scale=float(SCALE),
                            accum_out=sums[:, ui : ui + 1],
                        )
                        if ui == 1 and pending is not None:
                            emit_tail(pending)
                            pending = None
                        nks = kw // P
                        pts = []
                        for ks in range(nks):
                            pt_ps = ptpsum.tile(
                                [P, P], bf16, tag="ptps", name="pt_ps"
                            )
                            nc.tensor.transpose(
                                pt_ps[:], p_t[:, bass.ts(ks, P)], ident_bf[:]
                            )
                            pt_sb = ptpool.tile(
                                [P, P], bf16, tag="ptsb", name="pt_sb"
                            )
                            nc.vector.tensor_copy(pt_sb[:], pt_ps[:])
                            pts.append(pt_sb)
                        for ks in range(nks):
                            kt_idx = ku0 * 2 + ks
                            first = ui == 0 and ks == 0
                            last = ui == len(units) - 1 and ks == nks - 1
                            nc.tensor.matmul(
                                o_lo[:],
                                pts[ks][:],
                                xk_sb[:, kt_idx, 0:C],
                                start=first,
                                stop=last,
                            )
                            nc.tensor.matmul(
                                o_hi[:],
                                pts[ks][:],
                                xk_sb[:, kt_idx, C:E],
                                start=first,
                                stop=last,
                            )
                    nu = len(units)
                    return (j, r, nu, o_lo, o_hi, sums)

                pending = None
                for j, r in ((0, 0), (1, 0), (1, 1), (2, 0), (2, 1), (3, 0), (3, 1), (0, 1)):
                    pending = emit_block(j, r, pending)
                emit_tail(emit_tail_norm(pending), fine=True)
    _split_excess_waits(nc)
    return nc


def _build_masks(par):
    """16 mask slots [P, 256] (bf16 on the wire): slot j*4 + kui*2 + r covers
    key unit ku = 2*(j+1)-2+kui for q-rows of owned chunk j, row tile r."""
    chunks = _CHUNKS[par]
    m = np.zeros((P, 16, Q), np.float32)
    for j in range(4):
        nk = 2 * (j + 1)
        c = chunks[j]
        need = c + 1
        for kui in range(2):
            ku = nk - 2 + kui
            for r in range(2):
                slot = j * 4 + kui * 2 + r
                if ku < need - 1:
                    continue  # fully live, zero mask
                if ku == need - 1:
                    qpos = c * Q + r * P + np.arange(P)[:, None]
                    kpos = ku * Q + np.arange(Q)[None, :]
                    m[:, slot] = np.where(kpos <= qpos, 0.0, np.float32(NEG))
                else:
                    m[:, slot] = NEG
    return np.ascontiguousarray(m.reshape(P, 16 * Q))


def _host_inputs(x, W_Q, W_K, W_V):
    """Per-core input maps (host-side prep: chunk selection + W_Q^T W_K)."""
    import ml_dtypes

    bf = ml_dtypes.bfloat16
    x = np.ascontiguousarray(np.asarray(x, dtype=np.float32))
    wm = np.ascontiguousarray(
        (np.asarray(W_Q, np.float64).T @ np.asarray(W_K, np.float64)).astype(bf)
    )
    wvT = np.ascontiguousarray(np.asarray(W_V, np.float32).T.astype(bf))
    in_maps = []
    for c in range(NCORES):
        b, par = c // 2, c % 2
        xb = x[b]
        xq_rows = np.concatenate(
            [xb[ch * Q : (ch + 1) * Q] for ch in _CHUNKS[par]]
        )
        in_maps.append(
            {
                "xT": np.ascontiguousarray(xb.T.astype(bf)),
                "xq": np.ascontiguousarray(xq_rows.T.astype(bf)),
                "xk": np.ascontiguousarray(xb.astype(bf)),
                "wm": wm,
                "wv": wvT,
                "masks": _build_masks(par).astype(bf),
            }
        )
    return in_maps


def kernel(x, W_Q, W_K, W_V):
    from concourse.bass_utils import run_bass_kernel_spmd

    if "nc" not in _CACHE:
        _CACHE["nc"] = _build_program()
    nc = _CACHE["nc"]

    in_maps = _host_inputs(x, W_Q, W_K, W_V)
    res = run_bass_kernel_spmd(nc, in_maps, list(range(NCORES)))

    out = np.empty((B, S, E), np.float32)
    for c in range(NCORES):
        b, par = c // 2, c % 2
        o = res.results[c]["out"]  # [1024, 1024]
        for j, ch in enumerate(_CHUNKS[par]):
            out[b, ch * Q : (ch + 1) * Q] = o[j * Q : (j + 1) * Q]
    return out


# revision 44
# speedup vs baseline: 1.0197x; 1.0034x over previous
"""Causal single-head attention (B=4, S=2048, E=1024, fp32) on 8 TRN2 NeuronCores.

Sharding: data-parallel over batch (4) x 2-way causal-balanced query split at
256-row granularity.  The sequence stays in causal order on every core; core
(b, par) owns 256-row query chunks {0,3,4,7} (par=0) or {1,2,5,6} (par=1),
shipped separately as xq (x^T restricted to the owned query columns).  The
device program is identical on all 8 cores (SPMD): program q-block j attends
key units [0 : 2*(j+1)*256); the owned chunks are assigned to blocks sorted
by causal need (need(c) = c+1 256-key-units), which by construction satisfies
need in {sched-1, sched}, so only the last two key units of each block ever
carry a mask (triangular diagonal / all-dead / all-live), applied from
per-core mask data via an identity matmul into the score PSUM.

Algebra: both weight applications are hoisted off the attention inner loop.
scores = (x@M) @ x^T with M = W_Q^T W_K precomputed on the host (kills the
K projection), and out = (P @ x) @ W_V^T (kills the V projection): the device
accumulates O1^j = P_j @ x in PSUM, normalizes by the softmax row-sum during
the PSUM->SBUF copy, transposes O1 on the PE, and applies W_V^T once per
128-row query tile.

All matmul operands are bf16 (PE rate is identical to f32r at free >= 256,
but DMA, SBUF, and copy traffic halve; max rel err vs the f32 reference is
~5e-3, well inside the 2e-2 gate).  Scores and O1 accumulate in f32 PSUM.
DMAs are few and consolidated (the HWDGE ring is FIFO, so issue order =
transfer order; dep-chains cost ~2.2us per link in DGE restarts).  The
per-block tails (normalize, O1 transpose, W_V projection, store) are
software-pipelined behind the next block's first score group.
"""

import numpy as np

B, S, E = 4, 2048, 1024
P = 128          # partitions
C = 512
Q = 256          # query block granularity
NEG = -1e9
NCORES = 8
SCALE = 1.0 / np.sqrt(np.float32(E))

_CHUNKS = {0: (0, 3, 4, 7), 1: (1, 2, 5, 6)}   # owned 256-chunks per par

_CACHE = {}


def _install_drain_patch():
    """walrus in this env fits only 1 sync wait per CTRL_NO instruction; split
    the TileContext end-of-kernel drain waits across trailing SP nops."""
    import concourse.mybir as mybir
    import concourse.tile as tile
    from concourse.vector_clock import ScopedClock

    if getattr(tile.TileContext, "_drain_split_installed", False):
        return

    def _split_drain_and_barrier(self, tick_clock, wait_clock):
        drain_inst = self.nc.sync.drain()
        wait_clock.add_sem_waits(
            drain_inst.ins, ScopedClock({None: tick_clock.global_clock})
        )
        si = drain_inst.ins.sync_info
        waits = list(si.on_wait) if si and si.on_wait else []
        if len(waits) > 1:
            si.on_wait = waits[:1]
            rest = waits[1:]
            while rest:
                chunk, rest = rest[:1], rest[1:]
                nop = self.nc.sync.nop(nofuse=True, hint="drain_wait_split")
                nsi = nop.ins.sync_info
                if nsi is None:
                    nop.ins.sync_info = mybir.SyncInfo(on_wait=chunk, on_update=[])
                else:
                    nsi.on_wait = list(nsi.on_wait) + chunk

        self.nc.all_engine_barrier()
        assert self.sems is not None
        popped = self.nc._tile_sem_poison_stack.pop()
        assert popped is self._sem_poison
        self.nc.clear_and_free_semaphores(list(self.sems.allocated().values()))
        self.nc.all_engine_barrier()

    tile.TileContext._drain_and_barrier = _split_drain_and_barrier
    tile.TileContext._drain_split_installed = True


def _split_excess_waits(nc, limit=1):
    """walrus here fits only `limit` sync waits per instruction; move excess
    waits of every instruction onto injected same-engine NoOps placed directly
    before it (program order on the engine preserves the semantics)."""
    import copy

    import concourse.mybir as mybir

    template = None
    for f in nc.m.functions:
        for bb in f.blocks:
            for inst in bb.instructions:
                if type(inst).__name__ == "InstNoOp":
                    template = inst
                    break
            if template is not None:
                break
        if template is not None:
            break
    assert template is not None, "no InstNoOp template found"

    n = 0
    for f in nc.m.functions:
        for bb in f.blocks:
            new = []
            for inst in bb.instructions:
                si = inst.sync_info
                waits = list(si.on_wait) if si and si.on_wait else []
                if len(waits) > limit:
                    si.on_wait = waits[-limit:]
                    excess = waits[:-limit]
                    while excess:
                        chunk, excess = excess[:limit], excess[limit:]
                        nop = copy.copy(template)
                        nop.name = f"I-wsplit-{n}"
                        n += 1
                        nop.engine = inst.engine
                        nop.sync_info = mybir.SyncInfo(on_wait=chunk, on_update=[])
                        import bass_rust

                        nop.set_nosync_dependencies(
                            bass_rust.InstructionNameOrderedSet()
                        )
                        nop.set_sync_dependencies(
                            bass_rust.InstructionNameOrderedSet()
                        )
                        new.append(nop)
                new.append(inst)
            bb.instructions[:] = new
    return n


def _build_program():
    """One SPMD program; per-core behaviour differs only through input data."""
    import concourse.bass as bass
    import concourse.mybir as mybir
    import concourse.tile as tile
    from concourse.masks import make_identity

    _install_drain_patch()

    f32 = mybir.dt.float32
    f32r = mybir.dt.float32r
    bf16 = mybir.dt.bfloat16
    Act = mybir.ActivationFunctionType

    nc = bass.Bass(dynamic_dma_scratch_size=128)
    xT = nc.declare_dram_parameter("xT", [E, S], bf16, isOutput=False)
    xq = nc.declare_dram_parameter("xq", [E, 2 * C], bf16, isOutput=False)
    xk = nc.declare_dram_parameter("xk", [S, E], bf16, isOutput=False)
    wm = nc.declare_dram_parameter("wm", [E, E], bf16, isOutput=False)
    wv = nc.declare_dram_parameter("wv", [E, E], bf16, isOutput=False)
    masks = nc.declare_dram_parameter("masks", [P, 16 * Q], bf16, isOutput=False)
    out = nc.declare_dram_parameter("out", [2 * C, E], f32, isOutput=True)

    xT_r = xT.rearrange("(et p) s -> p et s", p=P)      # [128, 8, 2048]
    xq_r = xq.rearrange("(et p) q -> p et q", p=P)      # [128, 8, 1024]
    xk_r = xk.rearrange("(kt p) e -> p kt e", p=P)      # [128, 16, 1024]
    wm_r = wm.rearrange("(et p) d -> p et d", p=P)      # [128, 8, 1024]
    wv_r = wv.rearrange("(et p) d -> p et d", p=P)

    ET = E // P   # 8 contraction tiles
    DT = E // P   # 8 head-dim tiles
    KTiles = S // P  # 16 key tiles

    with tile.TileContext(nc) as tc:
        from contextlib import ExitStack

        with ExitStack() as ctx:
            big = ctx.enter_context(tc.tile_pool(name="big", bufs=1))
            mpool = ctx.enter_context(tc.tile_pool(name="mask", bufs=1))
            kvp = ctx.enter_context(tc.tile_pool(name="kv", bufs=1))
            ident = mpool.tile([P, P], f32)
            make_identity(nc, ident)
            ident_r = mpool.tile([P, P], f32r)
            nc.vector.tensor_copy(ident_r[:], ident[:])
            ident_bf = mpool.tile([P, P], bf16)
            nc.vector.tensor_copy(ident_bf[:], ident[:])
            masks_sb = mpool.tile([P, 16 * Q], bf16)
            zbias = mpool.tile([P, 1], f32)
            nc.vector.memset(zbias[:], 0.0)
            xk_sb = kvp.tile([P, KTiles, E], bf16, tag="xk")
            wv_sb = kvp.tile([P, ET, E], bf16, tag="wvf")
            xt_sb = big.tile([P, ET, S], bf16, tag="xt")    # x^T [e, s]
            gt_sb0 = big.tile([P, DT, C], bf16, tag="gt0")  # G^T [e, q] j0|j1
            gt_sb1 = big.tile([P, DT, C], bf16, tag="gt1")  # G^T [e, q] j2|j3

            # ---- G^T = M^T xq^T for the core's 1024 owned query columns ----
            with ExitStack() as pctx:
                wmp = pctx.enter_context(tc.tile_pool(name="wm", bufs=1))
                xqp = pctx.enter_context(tc.tile_pool(name="xq", bufs=1))
                gpsum = pctx.enter_context(
                    tc.tile_pool(name="gpsum", bufs=1, space="PSUM")
                )

                wm_sb = wmp.tile([P, ET, E], bf16, tag="wm")
                xq_sb = xqp.tile([P, ET, 2 * C], bf16, tag="xq")
                # Consolidated DMAs, no dep chains: the HWDGE ring is FIFO,
                # so issue order = transfer order at full bandwidth.  Chained
                # DMAs pay ~2.2us of DGE-restart latency per link.
                nc.sync.dma_start(wm_sb[:, 0, 0:C], wm_r[:, 0, 0:C])
                nc.sync.dma_start(xq_sb[:, 0, 0:C], xq_r[:, 0, 0:C])
                nc.sync.dma_start(wm_sb[:, 0, C:E], wm_r[:, 0, C:E])
                for et in range(1, ET):
                    nc.sync.dma_start(wm_sb[:, et, :], wm_r[:, et, :])
                    nc.sync.dma_start(
                        xq_sb[:, et, 0:C], xq_r[:, et, 0:C]
                    )
                nc.sync.dma_start(
                    xq_sb[:, :, C : 2 * C], xq_r[:, :, C : 2 * C]
                )
                # non-critical inputs, in first-use order
                nc.sync.dma_start(xt_sb[:, :, 0:C], xT_r[:, :, 0:C])
                nc.sync.dma_start(masks_sb[:], masks[:])
                nc.sync.dma_start(xk_sb[:, 0:4, :], xk_r[:, 0:4, :])
                nc.sync.dma_start(wv_sb[:], wv_r[:])
                nc.sync.dma_start(xt_sb[:, :, C : 2 * C], xT_r[:, :, C : 2 * C])
                nc.sync.dma_start(xk_sb[:, 4:8, :], xk_r[:, 4:8, :])
                nc.sync.dma_start(
                    xt_sb[:, :, 2 * C : 4 * C], xT_r[:, :, 2 * C : 4 * C]
                )
                nc.sync.dma_start(xk_sb[:, 8:16, :], xk_r[:, 8:16, :])

                for qb in range(2):
                    gps = [
                        gpsum.tile([P, C], f32, tag=f"gp{dt}", name=f"gp{qb}_{dt}")
                        for dt in range(DT)
                    ]
                    gdst = gt_sb0 if qb == 0 else gt_sb1
                    for dt in range(DT):
                        for et in range(ET):
                            nc.tensor.matmul(
                                gps[dt][:],
                                wm_sb[:, et, bass.ts(dt, P)],
                                xq_sb[:, et, bass.ts(qb, C)],
                                start=(et == 0),
                                stop=(et == ET - 1),
                            )
                        if dt == DT - 1:
                            nc.vector.tensor_copy(
                                gdst[:, dt, 0:C // 2], gps[dt][:, 0:C // 2]
                            )
                            nc.scalar.activation(
                                gdst[:, dt, C // 2 : C],
                                gps[dt][:, C // 2 : C],
                                Act.Copy,
                                scale=1.0,
                            )
                        elif dt % 2 == 0:
                            nc.vector.tensor_copy(gdst[:, dt, :], gps[dt][:])
                        else:
                            nc.scalar.activation(
                                gdst[:, dt, :], gps[dt][:], Act.Copy, scale=1.0
                            )

            # ---- attention: per q-block j (256 rows, r in {0,1}), key units
            # ku in [0, 2*(j+1)): scores -> exp -> P^T -> O1 += P^T-tile @ x,
            # normalize O1 by recip(rowsum) in the PSUM->SBUF copy, transpose
            # O1 on the PE, then out = O1 @ W_V^T.  Tails are pipelined into
            # the next block's first score group. ----
            with ExitStack() as actx:
                ppool = actx.enter_context(tc.tile_pool(name="p", bufs=4))
                ptpool = actx.enter_context(tc.tile_pool(name="pt", bufs=6))
                o1pool = actx.enter_context(tc.tile_pool(name="o1", bufs=2))
                o1tp = actx.enter_context(tc.tile_pool(name="o1t", bufs=2))
                obuf = actx.enter_context(tc.tile_pool(name="ob", bufs=4))
                stat = actx.enter_context(tc.tile_pool(name="stat", bufs=8))
                spsum = actx.enter_context(
                    tc.tile_pool(name="spsum", bufs=2, space="PSUM")
                )
                o1psum = actx.enter_context(
                    tc.tile_pool(name="o1ps", bufs=2, space="PSUM")
                )
                ptpsum = actx.enter_context(
                    tc.tile_pool(name="ptpsum", bufs=2, space="PSUM")
                )

                def emit_tail_norm(state):
                    j, r, nu, o_lo, o_hi, sums = state
                    stot = stat.tile([P, 1], f32, tag="stot", name="stot")
                    nc.vector.reduce_sum(
                        stot[:], sums[:, 0:nu], axis=mybir.AxisListType.X
                    )
                    recip = stat.tile([P, 1], f32, tag="recip", name="recip")
                    nc.vector.reciprocal(recip[:], stot[:])
                    # normalized O1 (softmax denominator applied here, so the
                    # final projection needs no epilogue scale), in quarter
                    # tiles split across Act and DVE so the first transpose
                    # input is ready fast
                    o1nq = []
                    for qq in range(4):
                        src_ps = o_lo if qq < 2 else o_hi
                        piece = o1pool.tile(
                            [P, Q], bf16, tag=f"o1nq{qq}", name="o1nq"
                        )
                        if qq % 2 == 0:
                            nc.scalar.activation(
                                piece[:],
                                src_ps[:, bass.ts(qq % 2, Q)],
                                Act.Copy,
                                scale=recip[:],
                            )
                        else:
                            nc.vector.tensor_scalar_mul(
                                piece[:], src_ps[:, bass.ts(qq % 2, Q)], recip[:]
                            )
                        o1nq.append(piece)
                    return (j, r, o1nq)

                def emit_tail(state, fine=False):
                    j, r, o1nq = state
                    o1ts = []
                    for et in range(ET):
                        piece = o1nq[et // 2]
                        tps = ptpsum.tile([P, P], bf16, tag="ptps", name="tps")
                        nc.tensor.transpose(
                            tps[:], piece[:, bass.ts(et % 2, P)], ident_bf[:]
                        )
                        o1t = o1tp.tile([P, P], bf16, tag=f"o1t{et}", name="o1t")
                        nc.vector.tensor_copy(o1t[:], tps[:])
                        o1ts.append(o1t)
                    nq = 4 if fine else 2
                    w = E // nq
                    for piece in range(nq):
                        fp = spsum.tile([P, w], f32, tag="s", name="fp")
                        for et in range(ET):
                            nc.tensor.matmul(
                                fp[:],
                                o1ts[et][:],
                                wv_sb[:, et, bass.ds(piece * w, w)],
                                start=(et == 0),
                                stop=(et == ET - 1),
                            )
                        obh = obuf.tile(
                            [P, w], f32, tag=f"ob{piece % 2}", name="obh"
                        )
                        if piece % 2 == 0:
                            nc.scalar.activation(
                                obh[:], fp[:], Act.Copy, scale=1.0
                            )
                        else:
                            nc.vector.tensor_copy(obh[:], fp[:])
                        nc.sync.dma_start(
                            out[bass.ds((j * 2 + r) * P, P), bass.ds(piece * w, w)],
                            obh[:],
                        )

                def emit_block(j, r, pending):
                    if pending is not None:
                        pending = emit_tail_norm(pending)
                    nk = 2 * (j + 1)
                    gsrc = gt_sb0 if j < 2 else gt_sb1
                    qcol0 = (j % 2) * Q + r * P
                    o_lo = o1psum.tile([P, C], f32, tag="olo", name="olo")
                    o_hi = o1psum.tile([P, C], f32, tag="ohi", name="ohi")
                    sums = stat.tile([P, 8], f32, tag="sums", name="sums")
                    # uniform 512-wide units (256-ku pairs); the two masked
                    # key units are adjacent, so the last unit carries one
                    # concatenated 512-wide mask slot
                    units = [(2 * u, 2) for u in range(nk // 2)]
                    for ui, (ku0, width) in enumerate(units):
                        kw = width * Q
                        s_t = spsum.tile([P, kw], f32, tag="s", name="s_t")
                        masked = ui == len(units) - 1
                        for dt in range(DT):
                            nc.tensor.matmul(
                                s_t[:],
                                gsrc[:, dt, bass.ds(qcol0, P)],
                                xt_sb[:, dt, bass.ds(ku0 * Q, kw)],
                                start=(dt == 0),
                                stop=(dt == DT - 1 and not masked),
                            )
                        if masked:
                            slot = j * 2 + r
                            nc.tensor.matmul(
                                s_t[:],
                                ident_bf[:],
                                masks_sb[:, bass.ds(slot * C, C)],
                                start=False,
                                stop=True,
                            )
                        p_t = ppool.tile([P, kw], bf16, tag="p", name="p_t")
                        nc.scalar.activation(
                            p_t[:],
                            s_t[:],
                            Act.Exp,
                            bias=zbias[:],
                            scale=float(SCALE),
                            accum_out=sums[:, ui : ui + 1],
                        )
                        if (
                            ui == (1 if len(units) > 1 else 0)
                            and pending is not None
                        ):
                            emit_tail(pending)
                            pending = None
                        nks = kw // P
                        pts = []
                        for ks in range(nks):
                            pt_ps = ptpsum.tile(
                                [P, P], bf16, tag="ptps", name="pt_ps"
                            )
                            nc.tensor.transpose(
                                pt_ps[:], p_t[:, bass.ts(ks, P)], ident_bf[:]
                            )
                            pt_sb = ptpool.tile(
                                [P, P], bf16, tag="ptsb", name="pt_sb"
                            )
                            nc.vector.tensor_copy(pt_sb[:], pt_ps[:])
                            pts.append(pt_sb)
                        for ks in range(nks):
                            kt_idx = ku0 * 2 + ks
                            first = ui == 0 and ks == 0
                            last = ui == len(units) - 1 and ks == nks - 1
                            nc.tensor.matmul(
                                o_lo[:],
                                pts[ks][:],
                                xk_sb[:, kt_idx, 0:C],
                                start=first,
                                stop=last,
                            )
                            nc.tensor.matmul(
                                o_hi[:],
                                pts[ks][:],
                                xk_sb[:, kt_idx, C:E],
                                start=first,
                                stop=last,
                            )
                    nu = len(units)
                    return (j, r, nu, o_lo, o_hi, sums)

                pending = None
                for j, r in ((0, 0), (1, 0), (1, 1), (2, 0), (2, 1), (3, 0), (3, 1), (0, 1)):
                    pending = emit_block(j, r, pending)
                emit_tail(emit_tail_norm(pending), fine=True)
    _split_excess_waits(nc)
    return nc


def _build_masks(par):
    """8 mask slots [P, 512] (bf16 on the wire): slot j*2 + r masks the last
    512-wide key unit (keys [(nk-2)*256, nk*256)) of owned chunk j, row r."""
    chunks = _CHUNKS[par]
    m = np.zeros((P, 8, 2 * Q), np.float32)
    for j in range(4):
        nk = 2 * (j + 1)
        c = chunks[j]
        for r in range(2):
            slot = j * 2 + r
            qpos = c * Q + r * P + np.arange(P)[:, None]
            kpos = (nk - 2) * Q + np.arange(2 * Q)[None, :]
            m[:, slot] = np.where(kpos <= qpos, 0.0, np.float32(NEG))
    return np.ascontiguousarray(m.reshape(P, 16 * Q))


def _host_inputs(x, W_Q, W_K, W_V):
    """Per-core input maps (host-side prep: chunk selection + W_Q^T W_K)."""
    import ml_dtypes

    bf = ml_dtypes.bfloat16
    x = np.ascontiguousarray(np.asarray(x, dtype=np.float32))
    wm = np.ascontiguousarray(
        (np.asarray(W_Q, np.float64).T @ np.asarray(W_K, np.float64)).astype(bf)
    )
    wvT = np.ascontiguousarray(np.asarray(W_V, np.float32).T.astype(bf))
    in_maps = []
    for c in range(NCORES):
        b, par = c // 2, c % 2
        xb = x[b]
        xq_rows = np.concatenate(
            [xb[ch * Q : (ch + 1) * Q] for ch in _CHUNKS[par]]
        )
        in_maps.append(
            {
                "xT": np.ascontiguousarray(xb.T.astype(bf)),
                "xq": np.ascontiguousarray(xq_rows.T.astype(bf)),
                "xk": np.ascontiguousarray(xb.astype(bf)),
                "wm": wm,
                "wv": wvT,
                "masks": _build_masks(par).astype(bf),
            }
        )
    return in_maps


def kernel(x, W_Q, W_K, W_V):
    from concourse.bass_utils import run_bass_kernel_spmd

    if "nc" not in _CACHE:
        _CACHE["nc"] = _build_program()
    nc = _CACHE["nc"]

    in_maps = _host_inputs(x, W_Q, W_K, W_V)
    res = run_bass_kernel_spmd(nc, in_maps, list(range(NCORES)))

    out = np.empty((B, S, E), np.float32)
    for c in range(NCORES):
        b, par = c // 2, c % 2
        o = res.results[c]["out"]  # [1024, 1024]
        for j, ch in enumerate(_CHUNKS[par]):
            out[b, ch * Q : (ch + 1) * Q] = o[j * Q : (j + 1) * Q]
    return out


# revision 47
# speedup vs baseline: 1.0300x; 1.0100x over previous
"""Causal single-head attention (B=4, S=2048, E=1024, fp32) on 8 TRN2 NeuronCores.

Sharding: data-parallel over batch (4) x 2-way causal-balanced query split at
256-row granularity.  The sequence stays in causal order on every core; core
(b, par) owns 256-row query chunks {0,3,4,7} (par=0) or {1,2,5,6} (par=1),
shipped separately as xq (x^T restricted to the owned query columns).  The
device program is identical on all 8 cores (SPMD): program q-block j attends
key units [0 : 2*(j+1)*256); the owned chunks are assigned to blocks sorted
by causal need (need(c) = c+1 256-key-units), which by construction satisfies
need in {sched-1, sched}, so only the last two key units of each block ever
carry a mask (triangular diagonal / all-dead / all-live), applied from
per-core mask data via an identity matmul into the score PSUM.

Algebra: both weight applications are hoisted off the attention inner loop.
scores = (x@M) @ x^T with M = W_Q^T W_K precomputed on the host (kills the
K projection), and out = (P @ x) @ W_V^T (kills the V projection): the device
accumulates O1^j = P_j @ x in PSUM, normalizes by the softmax row-sum during
the PSUM->SBUF copy, transposes O1 on the PE, and applies W_V^T once per
128-row query tile.

All matmul operands are bf16 (PE rate is identical to f32r at free >= 256,
but DMA, SBUF, and copy traffic halve; max rel err vs the f32 reference is
~5e-3, well inside the 2e-2 gate).  Scores and O1 accumulate in f32 PSUM.
DMAs are few and consolidated (the HWDGE ring is FIFO, so issue order =
transfer order; dep-chains cost ~2.2us per link in DGE restarts).  The
per-block tails (normalize, O1 transpose, W_V projection, store) are
software-pipelined behind the next block's first score group.
"""

import numpy as np

B, S, E = 4, 2048, 1024
P = 128          # partitions
C = 512
Q = 256          # query block granularity
NEG = -1e9
NCORES = 8
SCALE = 1.0 / np.sqrt(np.float32(E))

_CHUNKS = {0: (0, 3, 4, 7), 1: (1, 2, 5, 6)}   # owned 256-chunks per par

_CACHE = {}


def _install_drain_patch():
    """walrus in this env fits only 1 sync wait per CTRL_NO instruction; split
    the TileContext end-of-kernel drain waits across trailing SP nops."""
    import concourse.mybir as mybir
    import concourse.tile as tile
    from concourse.vector_clock import ScopedClock

    if getattr(tile.TileContext, "_drain_split_installed", False):
        return

    def _split_drain_and_barrier(self, tick_clock, wait_clock):
        drain_inst = self.nc.sync.drain()
        wait_clock.add_sem_waits(
            drain_inst.ins, ScopedClock({None: tick_clock.global_clock})
        )
        si = drain_inst.ins.sync_info
        waits = list(si.on_wait) if si and si.on_wait else []
        if len(waits) > 1:
            si.on_wait = waits[:1]
            rest = waits[1:]
            while rest:
                chunk, rest = rest[:1], rest[1:]
                nop = self.nc.sync.nop(nofuse=True, hint="drain_wait_split")
                nsi = nop.ins.sync_info
                if nsi is None:
                    nop.ins.sync_info = mybir.SyncInfo(on_wait=chunk, on_update=[])
                else:
                    nsi.on_wait = list(nsi.on_wait) + chunk

        self.nc.all_engine_barrier()
        assert self.sems is not None
        popped = self.nc._tile_sem_poison_stack.pop()
        assert popped is self._sem_poison
        self.nc.clear_and_free_semaphores(list(self.sems.allocated().values()))
        self.nc.all_engine_barrier()

    tile.TileContext._drain_and_barrier = _split_drain_and_barrier
    tile.TileContext._drain_split_installed = True


def _split_excess_waits(nc, limit=1):
    """walrus here fits only `limit` sync waits per instruction; move excess
    waits of every instruction onto injected same-engine NoOps placed directly
    before it (program order on the engine preserves the semantics)."""
    import copy

    import concourse.mybir as mybir

    template = None
    for f in nc.m.functions:
        for bb in f.blocks:
            for inst in bb.instructions:
                if type(inst).__name__ == "InstNoOp":
                    template = inst
                    break
            if template is not None:
                break
        if template is not None:
            break
    assert template is not None, "no InstNoOp template found"

    n = 0
    for f in nc.m.functions:
        for bb in f.blocks:
            new = []
            for inst in bb.instructions:
                si = inst.sync_info
                waits = list(si.on_wait) if si and si.on_wait else []
                if len(waits) > limit:
                    si.on_wait = waits[-limit:]
                    excess = waits[:-limit]
                    while excess:
                        chunk, excess = excess[:limit], excess[limit:]
                        nop = copy.copy(template)
                        nop.name = f"I-wsplit-{n}"
                        n += 1
                        nop.engine = inst.engine
                        nop.sync_info = mybir.SyncInfo(on_wait=chunk, on_update=[])
                        import bass_rust

                        nop.set_nosync_dependencies(
                            bass_rust.InstructionNameOrderedSet()
                        )
                        nop.set_sync_dependencies(
                            bass_rust.InstructionNameOrderedSet()
                        )
                        new.append(nop)
                new.append(inst)
            bb.instructions[:] = new
    return n


def _build_program():
    """One SPMD program; per-core behaviour differs only through input data."""
    import concourse.bass as bass
    import concourse.mybir as mybir
    import concourse.tile as tile
    from concourse.masks import make_identity

    _install_drain_patch()

    f32 = mybir.dt.float32
    f32r = mybir.dt.float32r
    bf16 = mybir.dt.bfloat16
    Act = mybir.ActivationFunctionType

    nc = bass.Bass(dynamic_dma_scratch_size=128)
    xT = nc.declare_dram_parameter("xT", [E, S], bf16, isOutput=False)
    xq = nc.declare_dram_parameter("xq", [E, 2 * C], bf16, isOutput=False)
    xk = nc.declare_dram_parameter("xk", [S, E], bf16, isOutput=False)
    wm = nc.declare_dram_parameter("wm", [E, E], bf16, isOutput=False)
    wv = nc.declare_dram_parameter("wv", [E, E], bf16, isOutput=False)
    masks = nc.declare_dram_parameter("masks", [P, 16 * Q], bf16, isOutput=False)
    out = nc.declare_dram_parameter("out", [2 * C, E], f32, isOutput=True)

    xT_r = xT.rearrange("(et p) s -> p et s", p=P)      # [128, 8, 2048]
    xq_r = xq.rearrange("(et p) q -> p et q", p=P)      # [128, 8, 1024]
    xk_r = xk.rearrange("(kt p) e -> p kt e", p=P)      # [128, 16, 1024]
    wm_r = wm.rearrange("(et p) d -> p et d", p=P)      # [128, 8, 1024]
    wv_r = wv.rearrange("(et p) d -> p et d", p=P)

    ET = E // P   # 8 contraction tiles
    DT = E // P   # 8 head-dim tiles
    KTiles = S // P  # 16 key tiles

    with tile.TileContext(nc) as tc:
        from contextlib import ExitStack

        with ExitStack() as ctx:
            big = ctx.enter_context(tc.tile_pool(name="big", bufs=1))
            mpool = ctx.enter_context(tc.tile_pool(name="mask", bufs=1))
            kvp = ctx.enter_context(tc.tile_pool(name="kv", bufs=1))
            ident = mpool.tile([P, P], f32)
            make_identity(nc, ident)
            ident_r = mpool.tile([P, P], f32r)
            nc.vector.tensor_copy(ident_r[:], ident[:])
            ident_bf = mpool.tile([P, P], bf16)
            nc.vector.tensor_copy(ident_bf[:], ident[:])
            masks_sb = mpool.tile([P, 16 * Q], bf16)
            zbias = mpool.tile([P, 1], f32)
            nc.vector.memset(zbias[:], 0.0)
            xk_sb = kvp.tile([P, KTiles, E], bf16, tag="xk")
            wv_sb = kvp.tile([P, ET, E], bf16, tag="wvf")
            xt_sb = big.tile([P, ET, S], bf16, tag="xt")    # x^T [e, s]
            gt_sb0 = big.tile([P, DT, C], bf16, tag="gt0")  # G^T [e, q] j0|j1
            gt_sb1 = big.tile([P, DT, C], bf16, tag="gt1")  # G^T [e, q] j2|j3

            # ---- G^T = M^T xq^T for the core's 1024 owned query columns ----
            with ExitStack() as pctx:
                wmp = pctx.enter_context(tc.tile_pool(name="wm", bufs=1))
                xqp = pctx.enter_context(tc.tile_pool(name="xq", bufs=1))
                gpsum = pctx.enter_context(
                    tc.tile_pool(name="gpsum", bufs=1, space="PSUM")
                )

                wm_sb = wmp.tile([P, ET, E], bf16, tag="wm")
                xq_sb = xqp.tile([P, ET, 2 * C], bf16, tag="xq")
                # Consolidated DMAs, no dep chains: the HWDGE ring is FIFO,
                # so issue order = transfer order at full bandwidth.  Chained
                # DMAs pay ~2.2us of DGE-restart latency per link.
                nc.sync.dma_start(wm_sb[:, 0, 0:C], wm_r[:, 0, 0:C])
                nc.sync.dma_start(xq_sb[:, 0, 0:C], xq_r[:, 0, 0:C])
                nc.sync.dma_start(wm_sb[:, 0, C:E], wm_r[:, 0, C:E])
                for et in range(1, ET):
                    nc.sync.dma_start(wm_sb[:, et, :], wm_r[:, et, :])
                    nc.sync.dma_start(
                        xq_sb[:, et, 0:C], xq_r[:, et, 0:C]
                    )
                nc.sync.dma_start(
                    xq_sb[:, :, C : 2 * C], xq_r[:, :, C : 2 * C]
                )
                # non-critical inputs, in first-use order
                nc.sync.dma_start(xt_sb[:, :, 0:C], xT_r[:, :, 0:C])
                nc.sync.dma_start(masks_sb[:], masks[:])
                nc.sync.dma_start(xk_sb[:, 0:4, :], xk_r[:, 0:4, :])
                nc.sync.dma_start(wv_sb[:], wv_r[:])
                nc.sync.dma_start(xt_sb[:, :, C : 2 * C], xT_r[:, :, C : 2 * C])
                nc.sync.dma_start(xk_sb[:, 4:8, :], xk_r[:, 4:8, :])
                nc.sync.dma_start(
                    xt_sb[:, :, 2 * C : 4 * C], xT_r[:, :, 2 * C : 4 * C]
                )
                nc.sync.dma_start(xk_sb[:, 8:16, :], xk_r[:, 8:16, :])

                for qb in range(2):
                    gps = [
                        gpsum.tile([P, C], f32, tag=f"gp{dt}", name=f"gp{qb}_{dt}")
                        for dt in range(DT)
                    ]
                    gdst = gt_sb0 if qb == 0 else gt_sb1
                    for dt in range(DT):
                        for et in range(ET):
                            nc.tensor.matmul(
                                gps[dt][:],
                                wm_sb[:, et, bass.ts(dt, P)],
                                xq_sb[:, et, bass.ts(qb, C)],
                                start=(et == 0),
                                stop=(et == ET - 1),
                            )
                        if dt == DT - 1:
                            nc.vector.tensor_copy(
                                gdst[:, dt, 0:C // 2], gps[dt][:, 0:C // 2]
                            )
                            nc.scalar.activation(
                                gdst[:, dt, C // 2 : C],
                                gps[dt][:, C // 2 : C],
                                Act.Copy,
                                scale=1.0,
                            )
                        elif dt % 2 == 0:
                            nc.vector.tensor_copy(gdst[:, dt, :], gps[dt][:])
                        else:
                            nc.scalar.activation(
                                gdst[:, dt, :], gps[dt][:], Act.Copy, scale=1.0
                            )

            # ---- attention: per q-block j (256 rows, r in {0,1}), key units
            # ku in [0, 2*(j+1)): scores -> exp -> P^T -> O1 += P^T-tile @ x,
            # normalize O1 by recip(rowsum) in the PSUM->SBUF copy, transpose
            # O1 on the PE, then out = O1 @ W_V^T.  Tails are pipelined into
            # the next block's first score group. ----
            with ExitStack() as actx:
                ppool = actx.enter_context(tc.tile_pool(name="p", bufs=4))
                ptpool = actx.enter_context(tc.tile_pool(name="pt", bufs=6))
                o1pool = actx.enter_context(tc.tile_pool(name="o1", bufs=2))
                o1tp = actx.enter_context(tc.tile_pool(name="o1t", bufs=2))
                obuf = actx.enter_context(tc.tile_pool(name="ob", bufs=4))
                stat = actx.enter_context(tc.tile_pool(name="stat", bufs=8))
                spsum = actx.enter_context(
                    tc.tile_pool(name="spsum", bufs=2, space="PSUM")
                )
                o1psum = actx.enter_context(
                    tc.tile_pool(name="o1ps", bufs=2, space="PSUM")
                )
                ptpsum = actx.enter_context(
                    tc.tile_pool(name="ptpsum", bufs=2, space="PSUM")
                )

                def emit_tail_norm(state):
                    j, r, nu, o_lo, o_hi, sums = state
                    stot = stat.tile([P, 1], f32, tag="stot", name="stot")
                    nc.vector.reduce_sum(
                        stot[:], sums[:, 0:nu], axis=mybir.AxisListType.X
                    )
                    recip = stat.tile([P, 1], f32, tag="recip", name="recip")
                    nc.vector.reciprocal(recip[:], stot[:])
                    # normalized O1 (softmax denominator applied here, so the
                    # final projection needs no epilogue scale), in quarter
                    # tiles split across Act and DVE so the first transpose
                    # input is ready fast
                    o1nq = []
                    for qq in range(4):
                        src_ps = o_lo if qq < 2 else o_hi
                        piece = o1pool.tile(
                            [P, Q], bf16, tag=f"o1nq{qq}", name="o1nq"
                        )
                        if qq % 2 == 1:
                            nc.scalar.activation(
                                piece[:],
                                src_ps[:, bass.ts(qq % 2, Q)],
                                Act.Copy,
                                scale=recip[:],
                            )
                        else:
                            nc.vector.tensor_scalar_mul(
                                piece[:], src_ps[:, bass.ts(qq % 2, Q)], recip[:]
                            )
                        o1nq.append(piece)
                    return (j, r, o1nq)

                def emit_tail(state, fine=False):
                    j, r, o1nq = state
                    o1ts = []
                    for et in range(ET):
                        piece = o1nq[et // 2]
                        tps = ptpsum.tile([P, P], bf16, tag="ptps", name="tps")
                        nc.tensor.transpose(
                            tps[:], piece[:, bass.ts(et % 2, P)], ident_bf[:]
                        )
                        o1t = o1tp.tile([P, P], bf16, tag=f"o1t{et}", name="o1t")
                        nc.vector.tensor_copy(o1t[:], tps[:])
                        o1ts.append(o1t)
                    nq = 4 if fine else 2
                    w = E // nq
                    for piece in range(nq):
                        fp = spsum.tile([P, w], f32, tag="s", name="fp")
                        for et in range(ET):
                            nc.tensor.matmul(
                                fp[:],
                                o1ts[et][:],
                                wv_sb[:, et, bass.ds(piece * w, w)],
                                start=(et == 0),
                                stop=(et == ET - 1),
                            )
                        obh = obuf.tile(
                            [P, w], f32, tag=f"ob{piece % 2}", name="obh"
                        )
                        if piece % 2 == 0:
                            nc.scalar.activation(
                                obh[:], fp[:], Act.Copy, scale=1.0
                            )
                        else:
                            nc.vector.tensor_copy(obh[:], fp[:])
                        nc.sync.dma_start(
                            out[bass.ds((j * 2 + r) * P, P), bass.ds(piece * w, w)],
                            obh[:],
                        )

                def emit_block(j, r, pending):
                    if pending is not None:
                        pending = emit_tail_norm(pending)
                    nk = 2 * (j + 1)
                    gsrc = gt_sb0 if j < 2 else gt_sb1
                    qcol0 = (j % 2) * Q + r * P
                    o_lo = o1psum.tile([P, C], f32, tag="olo", name="olo")
                    o_hi = o1psum.tile([P, C], f32, tag="ohi", name="ohi")
                    sums = stat.tile([P, 8], f32, tag="sums", name="sums")
                    # uniform 512-wide units (256-ku pairs); the two masked
                    # key units are adjacent, so the last unit carries one
                    # concatenated 512-wide mask slot
                    units = [(2 * u, 2) for u in range(nk // 2)]
                    for ui, (ku0, width) in enumerate(units):
                        kw = width * Q
                        s_t = spsum.tile([P, kw], f32, tag="s", name="s_t")
                        masked = ui == len(units) - 1
                        for dt in range(DT):
                            nc.tensor.matmul(
                                s_t[:],
                                gsrc[:, dt, bass.ds(qcol0, P)],
                                xt_sb[:, dt, bass.ds(ku0 * Q, kw)],
                                start=(dt == 0),
                                stop=(dt == DT - 1 and not masked),
                            )
                        if masked:
                            slot = j * 2 + r
                            nc.tensor.matmul(
                                s_t[:],
                                ident_bf[:],
                                masks_sb[:, bass.ds(slot * C, C)],
                                start=False,
                                stop=True,
                            )
                        p_t = ppool.tile([P, kw], bf16, tag="p", name="p_t")
                        nc.scalar.activation(
                            p_t[:],
                            s_t[:],
                            Act.Exp,
                            bias=zbias[:],
                            scale=float(SCALE),
                            accum_out=sums[:, ui : ui + 1],
                        )
                        if (
                            ui == (1 if len(units) > 1 else 0)
                            and pending is not None
                        ):
                            emit_tail(pending)
                            pending = None
                        nks = kw // P
                        pts = []
                        for ks in range(nks):
                            pt_ps = ptpsum.tile(
                                [P, P], bf16, tag="ptps", name="pt_ps"
                            )
                            nc.tensor.transpose(
                                pt_ps[:], p_t[:, bass.ts(ks, P)], ident_bf[:]
                            )
                            pt_sb = ptpool.tile(
                                [P, P], bf16, tag="ptsb", name="pt_sb"
                            )
                            nc.vector.tensor_copy(pt_sb[:], pt_ps[:])
                            pts.append(pt_sb)
                        for ks in range(nks):
                            kt_idx = ku0 * 2 + ks
                            first = ui == 0 and ks == 0
                            last = ui == len(units) - 1 and ks == nks - 1
                            nc.tensor.matmul(
                                o_lo[:],
                                pts[ks][:],
                                xk_sb[:, kt_idx, 0:C],
                                start=first,
                                stop=last,
                            )
                            nc.tensor.matmul(
                                o_hi[:],
                                pts[ks][:],
                                xk_sb[:, kt_idx, C:E],
                                start=first,
                                stop=last,
                            )
                    nu = len(units)
                    return (j, r, nu, o_lo, o_hi, sums)

                pending = None
                for j, r in ((0, 0), (1, 0), (1, 1), (2, 0), (2, 1), (3, 0), (3, 1), (0, 1)):
                    pending = emit_block(j, r, pending)
                emit_tail(emit_tail_norm(pending), fine=True)
    _split_excess_waits(nc)
    return nc


def _build_masks(par):
    """8 mask slots [P, 512] (bf16 on the wire): slot j*2 + r masks the last
    512-wide key unit (keys [(nk-2)*256, nk*256)) of owned chunk j, row r."""
    chunks = _CHUNKS[par]
    m = np.zeros((P, 8, 2 * Q), np.float32)
    for j in range(4):
        nk = 2 * (j + 1)
        c = chunks[j]
        for r in range(2):
            slot = j * 2 + r
            qpos = c * Q + r * P + np.arange(P)[:, None]
            kpos = (nk - 2) * Q + np.arange(2 * Q)[None, :]
            m[:, slot] = np.where(kpos <= qpos, 0.0, np.float32(NEG))
    return np.ascontiguousarray(m.reshape(P, 16 * Q))


def _host_inputs(x, W_Q, W_K, W_V):
    """Per-core input maps (host-side prep: chunk selection + W_Q^T W_K)."""
    import ml_dtypes

    bf = ml_dtypes.bfloat16
    x = np.ascontiguousarray(np.asarray(x, dtype=np.float32))
    wm = np.ascontiguousarray(
        (np.asarray(W_Q, np.float64).T @ np.asarray(W_K, np.float64)).astype(bf)
    )
    wvT = np.ascontiguousarray(np.asarray(W_V, np.float32).T.astype(bf))
    in_maps = []
    for c in range(NCORES):
        b, par = c // 2, c % 2
        xb = x[b]
        xq_rows = np.concatenate(
            [xb[ch * Q : (ch + 1) * Q] for ch in _CHUNKS[par]]
        )
        in_maps.append(
            {
                "xT": np.ascontiguousarray(xb.T.astype(bf)),
                "xq": np.ascontiguousarray(xq_rows.T.astype(bf)),
                "xk": np.ascontiguousarray(xb.astype(bf)),
                "wm": wm,
                "wv": wvT,
                "masks": _build_masks(par).astype(bf),
            }
        )
    return in_maps


def kernel(x, W_Q, W_K, W_V):
    from concourse.bass_utils import run_bass_kernel_spmd

    if "nc" not in _CACHE:
        _CACHE["nc"] = _build_program()
    nc = _CACHE["nc"]

    in_maps = _host_inputs(x, W_Q, W_K, W_V)
    res = run_bass_kernel_spmd(nc, in_maps, list(range(NCORES)))

    out = np.empty((B, S, E), np.float32)
    for c in range(NCORES):
        b, par = c // 2, c % 2
        o = res.results[c]["out"]  # [1024, 1024]
        for j, ch in enumerate(_CHUNKS[par]):
            out[b, ch * Q : (ch + 1) * Q] = o[j * Q : (j + 1) * Q]
    return out


# revision 55
# speedup vs baseline: 1.0338x; 1.0038x over previous
"""Causal single-head attention (B=4, S=2048, E=1024, fp32) on 8 TRN2 NeuronCores.

Sharding: data-parallel over batch (4) x 2-way causal-balanced query split at
256-row granularity.  The sequence stays in causal order on every core; core
(b, par) owns 256-row query chunks {0,3,4,7} (par=0) or {1,2,5,6} (par=1),
shipped separately as xq (x^T restricted to the owned query columns).  The
device program is identical on all 8 cores (SPMD): program q-block j attends
key units [0 : 2*(j+1)*256); the owned chunks are assigned to blocks sorted
by causal need (need(c) = c+1 256-key-units), which by construction satisfies
need in {sched-1, sched}, so only the last two key units of each block ever
carry a mask (triangular diagonal / all-dead / all-live), applied from
per-core mask data via an identity matmul into the score PSUM.

Algebra: both weight applications are hoisted off the attention inner loop.
scores = (x@M) @ x^T with M = W_Q^T W_K precomputed on the host (kills the
K projection), and out = (P @ x) @ W_V^T (kills the V projection): the device
accumulates O1^j = P_j @ x in PSUM, normalizes by the softmax row-sum during
the PSUM->SBUF copy, transposes O1 on the PE, and applies W_V^T once per
128-row query tile.

All matmul operands are bf16 (PE rate is identical to f32r at free >= 256,
but DMA, SBUF, and copy traffic halve; max rel err vs the f32 reference is
~5e-3, well inside the 2e-2 gate).  Scores and O1 accumulate in f32 PSUM.
DMAs are few and consolidated (the HWDGE ring is FIFO, so issue order =
transfer order; dep-chains cost ~2.2us per link in DGE restarts).  The
per-block tails (normalize, O1 transpose, W_V projection, store) are
software-pipelined behind the next block's first score group.
"""

import numpy as np

B, S, E = 4, 2048, 1024
P = 128          # partitions
C = 512
Q = 256          # query block granularity
NEG = -1e9
NCORES = 8
SCALE = 1.0 / np.sqrt(np.float32(E))

_CHUNKS = {0: (0, 3, 4, 7), 1: (1, 2, 5, 6)}   # owned 256-chunks per par

_CACHE = {}


def _install_drain_patch():
    """walrus in this env fits only 1 sync wait per CTRL_NO instruction; split
    the TileContext end-of-kernel drain waits across trailing SP nops."""
    import concourse.mybir as mybir
    import concourse.tile as tile
    from concourse.vector_clock import ScopedClock

    if getattr(tile.TileContext, "_drain_split_installed", False):
        return

    def _split_drain_and_barrier(self, tick_clock, wait_clock):
        drain_inst = self.nc.sync.drain()
        wait_clock.add_sem_waits(
            drain_inst.ins, ScopedClock({None: tick_clock.global_clock})
        )
        si = drain_inst.ins.sync_info
        waits = list(si.on_wait) if si and si.on_wait else []
        if len(waits) > 1:
            si.on_wait = waits[:1]
            rest = waits[1:]
            while rest:
                chunk, rest = rest[:1], rest[1:]
                nop = self.nc.sync.nop(nofuse=True, hint="drain_wait_split")
                nsi = nop.ins.sync_info
                if nsi is None:
                    nop.ins.sync_info = mybir.SyncInfo(on_wait=chunk, on_update=[])
                else:
                    nsi.on_wait = list(nsi.on_wait) + chunk

        self.nc.all_engine_barrier()
        assert self.sems is not None
        popped = self.nc._tile_sem_poison_stack.pop()
        assert popped is self._sem_poison
        self.nc.clear_and_free_semaphores(list(self.sems.allocated().values()))
        self.nc.all_engine_barrier()

    tile.TileContext._drain_and_barrier = _split_drain_and_barrier
    tile.TileContext._drain_split_installed = True


def _split_excess_waits(nc, limit=1):
    """walrus here fits only `limit` sync waits per instruction; move excess
    waits of every instruction onto injected same-engine NoOps placed directly
    before it (program order on the engine preserves the semantics)."""
    import copy

    import concourse.mybir as mybir

    template = None
    for f in nc.m.functions:
        for bb in f.blocks:
            for inst in bb.instructions:
                if type(inst).__name__ == "InstNoOp":
                    template = inst
                    break
            if template is not None:
                break
        if template is not None:
            break
    assert template is not None, "no InstNoOp template found"

    n = 0
    for f in nc.m.functions:
        for bb in f.blocks:
            new = []
            for inst in bb.instructions:
                si = inst.sync_info
                waits = list(si.on_wait) if si and si.on_wait else []
                if len(waits) > limit:
                    si.on_wait = waits[-limit:]
                    excess = waits[:-limit]
                    while excess:
                        chunk, excess = excess[:limit], excess[limit:]
                        nop = copy.copy(template)
                        nop.name = f"I-wsplit-{n}"
                        n += 1
                        nop.engine = inst.engine
                        nop.sync_info = mybir.SyncInfo(on_wait=chunk, on_update=[])
                        import bass_rust

                        nop.set_nosync_dependencies(
                            bass_rust.InstructionNameOrderedSet()
                        )
                        nop.set_sync_dependencies(
                            bass_rust.InstructionNameOrderedSet()
                        )
                        new.append(nop)
                new.append(inst)
            bb.instructions[:] = new
    return n


def _build_program():
    """One SPMD program; per-core behaviour differs only through input data."""
    import concourse.bass as bass
    import concourse.mybir as mybir
    import concourse.tile as tile
    from concourse.masks import make_identity

    _install_drain_patch()

    f32 = mybir.dt.float32
    f32r = mybir.dt.float32r
    bf16 = mybir.dt.bfloat16
    Act = mybir.ActivationFunctionType

    nc = bass.Bass(dynamic_dma_scratch_size=128)
    xT = nc.declare_dram_parameter("xT", [E, S], bf16, isOutput=False)
    xq = nc.declare_dram_parameter("xq", [E, 2 * C], bf16, isOutput=False)
    xk = nc.declare_dram_parameter("xk", [S, E], bf16, isOutput=False)
    wm = nc.declare_dram_parameter("wm", [E, E], bf16, isOutput=False)
    wv = nc.declare_dram_parameter("wv", [E, E], bf16, isOutput=False)
    masks = nc.declare_dram_parameter("masks", [P, 16 * Q], bf16, isOutput=False)
    out = nc.declare_dram_parameter("out", [2 * C, E], f32, isOutput=True)

    xT_r = xT.rearrange("(et p) s -> p et s", p=P)      # [128, 8, 2048]
    xq_r = xq.rearrange("(et p) q -> p et q", p=P)      # [128, 8, 1024]
    xk_r = xk.rearrange("(kt p) e -> p kt e", p=P)      # [128, 16, 1024]
    wm_r = wm.rearrange("(et p) d -> p et d", p=P)      # [128, 8, 1024]
    wv_r = wv.rearrange("(et p) d -> p et d", p=P)

    ET = E // P   # 8 contraction tiles
    DT = E // P   # 8 head-dim tiles
    KTiles = S // P  # 16 key tiles

    with tile.TileContext(nc) as tc:
        from contextlib import ExitStack

        with ExitStack() as ctx:
            big = ctx.enter_context(tc.tile_pool(name="big", bufs=1))
            mpool = ctx.enter_context(tc.tile_pool(name="mask", bufs=1))
            kvp = ctx.enter_context(tc.tile_pool(name="kv", bufs=1))
            ident = mpool.tile([P, P], f32)
            make_identity(nc, ident)
            ident_r = mpool.tile([P, P], f32r)
            nc.vector.tensor_copy(ident_r[:], ident[:])
            ident_bf = mpool.tile([P, P], bf16)
            nc.vector.tensor_copy(ident_bf[:], ident[:])
            masks_sb = mpool.tile([P, 16 * Q], bf16)
            zbias = mpool.tile([P, 1], f32)
            nc.vector.memset(zbias[:], 0.0)
            xk_sb = kvp.tile([P, KTiles, E], bf16, tag="xk")
            wv_sb = kvp.tile([P, ET, E], bf16, tag="wvf")
            xt_sb = big.tile([P, ET, S], bf16, tag="xt")    # x^T [e, s]
            gt_sb0 = big.tile([P, DT, C], bf16, tag="gt0")  # G^T [e, q] j0|j1
            gt_sb1 = big.tile([P, DT, C], bf16, tag="gt1")  # G^T [e, q] j2|j3

            # ---- G^T = M^T xq^T for the core's 1024 owned query columns ----
            with ExitStack() as pctx:
                wmp = pctx.enter_context(tc.tile_pool(name="wm", bufs=1))
                xqp = pctx.enter_context(tc.tile_pool(name="xq", bufs=1))
                gpsum = pctx.enter_context(
                    tc.tile_pool(name="gpsum", bufs=1, space="PSUM")
                )

                wm_sb = wmp.tile([P, ET, E], bf16, tag="wm")
                xq_sb = xqp.tile([P, ET, 2 * C], bf16, tag="xq")
                # Consolidated DMAs, no dep chains: the HWDGE ring is FIFO,
                # so issue order = transfer order at full bandwidth.  Chained
                # DMAs pay ~2.2us of DGE-restart latency per link.
                nc.sync.dma_start(wm_sb[:, 0, 0:C], wm_r[:, 0, 0:C])
                nc.sync.dma_start(xq_sb[:, 0, 0:C], xq_r[:, 0, 0:C])
                nc.sync.dma_start(wm_sb[:, 0, C:E], wm_r[:, 0, C:E])
                for et in range(1, ET):
                    nc.sync.dma_start(wm_sb[:, et, :], wm_r[:, et, :])
                    nc.sync.dma_start(
                        xq_sb[:, et, 0:C], xq_r[:, et, 0:C]
                    )
                nc.sync.dma_start(
                    xq_sb[:, :, C : 2 * C], xq_r[:, :, C : 2 * C]
                )
                # non-critical inputs, in first-use order
                nc.sync.dma_start(xt_sb[:, :, 0:C], xT_r[:, :, 0:C])
                nc.sync.dma_start(masks_sb[:], masks[:])
                nc.sync.dma_start(xk_sb[:, 0:4, :], xk_r[:, 0:4, :])
                nc.sync.dma_start(wv_sb[:], wv_r[:])
                nc.sync.dma_start(xt_sb[:, :, C : 2 * C], xT_r[:, :, C : 2 * C])
                nc.sync.dma_start(xk_sb[:, 4:8, :], xk_r[:, 4:8, :])
                nc.sync.dma_start(
                    xt_sb[:, :, 2 * C : 4 * C], xT_r[:, :, 2 * C : 4 * C]
                )
                nc.sync.dma_start(xk_sb[:, 8:16, :], xk_r[:, 8:16, :])

                for qb in range(2):
                    gps = [
                        gpsum.tile([P, C], f32, tag=f"gp{dt}", name=f"gp{qb}_{dt}")
                        for dt in range(DT)
                    ]
                    gdst = gt_sb0 if qb == 0 else gt_sb1
                    for dt in range(DT):
                        for et in range(ET):
                            nc.tensor.matmul(
                                gps[dt][:],
                                wm_sb[:, et, bass.ts(dt, P)],
                                xq_sb[:, et, bass.ts(qb, C)],
                                start=(et == 0),
                                stop=(et == ET - 1),
                            )
                        if dt == DT - 1:
                            nc.vector.tensor_copy(gdst[:, dt, :], gps[dt][:])
                        elif dt % 2 == 0:
                            nc.vector.tensor_copy(gdst[:, dt, :], gps[dt][:])
                        else:
                            nc.scalar.activation(
                                gdst[:, dt, :], gps[dt][:], Act.Copy, scale=1.0
                            )

            # ---- attention: per q-block j (256 rows, r in {0,1}), key units
            # ku in [0, 2*(j+1)): scores -> exp -> P^T -> O1 += P^T-tile @ x,
            # normalize O1 by recip(rowsum) in the PSUM->SBUF copy, transpose
            # O1 on the PE, then out = O1 @ W_V^T.  Tails are pipelined into
            # the next block's first score group. ----
            with ExitStack() as actx:
                ppool = actx.enter_context(tc.tile_pool(name="p", bufs=4))
                ptpool = actx.enter_context(tc.tile_pool(name="pt", bufs=6))
                o1pool = actx.enter_context(tc.tile_pool(name="o1", bufs=2))
                o1tp = actx.enter_context(tc.tile_pool(name="o1t", bufs=2))
                obuf = actx.enter_context(tc.tile_pool(name="ob", bufs=4))
                stat = actx.enter_context(tc.tile_pool(name="stat", bufs=8))
                spsum = actx.enter_context(
                    tc.tile_pool(name="spsum", bufs=2, space="PSUM")
                )
                o1psum = actx.enter_context(
                    tc.tile_pool(name="o1ps", bufs=2, space="PSUM")
                )
                ptpsum = actx.enter_context(
                    tc.tile_pool(name="ptpsum", bufs=2, space="PSUM")
                )

                def emit_tail_norm(state):
                    j, r, nu, o_lo, o_hi, sums = state
                    stot = stat.tile([P, 1], f32, tag="stot", name="stot")
                    nc.vector.reduce_sum(
                        stot[:], sums[:, 0:nu], axis=mybir.AxisListType.X
                    )
                    recip = stat.tile([P, 1], f32, tag="recip", name="recip")
                    nc.vector.reciprocal(recip[:], stot[:])
                    # normalized O1 (softmax denominator applied here, so the
                    # final projection needs no epilogue scale), in quarter
                    # tiles split across Act and DVE so the first transpose
                    # input is ready fast
                    o1nq = []
                    for qq in range(4):
                        src_ps = o_lo if qq < 2 else o_hi
                        piece = o1pool.tile(
                            [P, Q], bf16, tag=f"o1nq{qq}", name="o1nq"
                        )
                        if qq % 2 == 1:
                            nc.scalar.activation(
                                piece[:],
                                src_ps[:, bass.ts(qq % 2, Q)],
                                Act.Copy,
                                scale=recip[:],
                            )
                        else:
                            nc.vector.tensor_scalar_mul(
                                piece[:], src_ps[:, bass.ts(qq % 2, Q)], recip[:]
                            )
                        o1nq.append(piece)
                    return (j, r, o1nq)

                def emit_tail(state, fine=False):
                    j, r, o1nq = state
                    o1ts = []
                    for et in range(ET):
                        piece = o1nq[et // 2]
                        tps = ptpsum.tile([P, P], bf16, tag="ptps", name="tps")
                        nc.tensor.transpose(
                            tps[:], piece[:, bass.ts(et % 2, P)], ident_bf[:]
                        )
                        o1t = o1tp.tile([P, P], bf16, tag=f"o1t{et}", name="o1t")
                        nc.vector.tensor_copy(o1t[:], tps[:])
                        o1ts.append(o1t)
                    nq = 4 if fine else 2
                    w = E // nq
                    for piece in range(nq):
                        fp = spsum.tile([P, w], f32, tag="s", name="fp")
                        for et in range(ET):
                            nc.tensor.matmul(
                                fp[:],
                                o1ts[et][:],
                                wv_sb[:, et, bass.ds(piece * w, w)],
                                start=(et == 0),
                                stop=(et == ET - 1),
                            )
                        obh = obuf.tile(
                            [P, w], f32, tag=f"ob{piece % 2}", name="obh"
                        )
                        if piece % 2 == 1:
                            nc.scalar.activation(
                                obh[:], fp[:], Act.Copy, scale=1.0
                            )
                        else:
                            nc.vector.tensor_copy(obh[:], fp[:])
                        nc.sync.dma_start(
                            out[bass.ds((j * 2 + r) * P, P), bass.ds(piece * w, w)],
                            obh[:],
                        )

                def emit_block(j, r, pending):
                    if pending is not None:
                        pending = emit_tail_norm(pending)
                    nk = 2 * (j + 1)
                    gsrc = gt_sb0 if j < 2 else gt_sb1
                    qcol0 = (j % 2) * Q + r * P
                    o_lo = o1psum.tile([P, C], f32, tag="olo", name="olo")
                    o_hi = o1psum.tile([P, C], f32, tag="ohi", name="ohi")
                    sums = stat.tile([P, 8], f32, tag="sums", name="sums")
                    # uniform 512-wide units (256-ku pairs); the two masked
                    # key units are adjacent, so the last unit carries one
                    # concatenated 512-wide mask slot
                    units = [(2 * u, 2) for u in range(nk // 2)]
                    for ui, (ku0, width) in enumerate(units):
                        kw = width * Q
                        s_t = spsum.tile([P, kw], f32, tag="s", name="s_t")
                        masked = ui == len(units) - 1
                        for dt in range(DT):
                            nc.tensor.matmul(
                                s_t[:],
                                gsrc[:, dt, bass.ds(qcol0, P)],
                                xt_sb[:, dt, bass.ds(ku0 * Q, kw)],
                                start=(dt == 0),
                                stop=(dt == DT - 1 and not masked),
                            )
                        if masked:
                            slot = j * 2 + r
                            nc.tensor.matmul(
                                s_t[:],
                                ident_bf[:],
                                masks_sb[:, bass.ds(slot * C, C)],
                                start=False,
                                stop=True,
                            )
                        p_t = ppool.tile([P, kw], bf16, tag="p", name="p_t")
                        nc.scalar.activation(
                            p_t[:],
                            s_t[:],
                            Act.Exp,
                            bias=zbias[:],
                            scale=float(SCALE),
                            accum_out=sums[:, ui : ui + 1],
                        )
                        if (
                            ui == (1 if len(units) > 1 else 0)
                            and pending is not None
                        ):
                            emit_tail(pending)
                            pending = None
                        nks = kw // P
                        pts = []
                        for ks in range(nks):
                            pt_ps = ptpsum.tile(
                                [P, P], bf16, tag="ptps", name="pt_ps"
                            )
                            nc.tensor.transpose(
                                pt_ps[:], p_t[:, bass.ts(ks, P)], ident_bf[:]
                            )
                            pt_sb = ptpool.tile(
                                [P, P], bf16, tag="ptsb", name="pt_sb"
                            )
                            nc.vector.tensor_copy(pt_sb[:], pt_ps[:])
                            pts.append(pt_sb)
                        for ks in range(nks):
                            kt_idx = ku0 * 2 + ks
                            first = ui == 0 and ks == 0
                            last = ui == len(units) - 1 and ks == nks - 1
                            nc.tensor.matmul(
                                o_lo[:],
                                pts[ks][:],
                                xk_sb[:, kt_idx, 0:C],
                                start=first,
                                stop=last,
                            )
                            nc.tensor.matmul(
                                o_hi[:],
                                pts[ks][:],
                                xk_sb[:, kt_idx, C:E],
                                start=first,
                                stop=last,
                            )
                    nu = len(units)
                    return (j, r, nu, o_lo, o_hi, sums)

                pending = None
                for j, r in ((0, 0), (1, 0), (1, 1), (2, 0), (2, 1), (3, 0), (3, 1), (0, 1)):
                    pending = emit_block(j, r, pending)
                emit_tail(emit_tail_norm(pending), fine=True)
    _split_excess_waits(nc)
    return nc


def _build_masks(par):
    """8 mask slots [P, 512] (bf16 on the wire): slot j*2 + r masks the last
    512-wide key unit (keys [(nk-2)*256, nk*256)) of owned chunk j, row r."""
    chunks = _CHUNKS[par]
    m = np.zeros((P, 8, 2 * Q), np.float32)
    for j in range(4):
        nk = 2 * (j + 1)
        c = chunks[j]
        for r in range(2):
            slot = j * 2 + r
            qpos = c * Q + r * P + np.arange(P)[:, None]
            kpos = (nk - 2) * Q + np.arange(2 * Q)[None, :]
            m[:, slot] = np.where(kpos <= qpos, 0.0, np.float32(NEG))
    return np.ascontiguousarray(m.reshape(P, 16 * Q))


def _host_inputs(x, W_Q, W_K, W_V):
    """Per-core input maps (host-side prep: chunk selection + W_Q^T W_K)."""
    import ml_dtypes

    bf = ml_dtypes.bfloat16
    x = np.ascontiguousarray(np.asarray(x, dtype=np.float32))
    wm = np.ascontiguousarray(
        (np.asarray(W_Q, np.float64).T @ np.asarray(W_K, np.float64)).astype(bf)
    )
    wvT = np.ascontiguousarray(np.asarray(W_V, np.float32).T.astype(bf))
    in_maps = []
    for c in range(NCORES):
        b, par = c // 2, c % 2
        xb = x[b]
        xq_rows = np.concatenate(
            [xb[ch * Q : (ch + 1) * Q] for ch in _CHUNKS[par]]
        )
        in_maps.append(
            {
                "xT": np.ascontiguousarray(xb.T.astype(bf)),
                "xq": np.ascontiguousarray(xq_rows.T.astype(bf)),
                "xk": np.ascontiguousarray(xb.astype(bf)),
                "wm": wm,
                "wv": wvT,
                "masks": _build_masks(par).astype(bf),
            }
        )
    return in_maps


def kernel(x, W_Q, W_K, W_V):
    from concourse.bass_utils import run_bass_kernel_spmd

    if "nc" not in _CACHE:
        _CACHE["nc"] = _build_program()
    nc = _CACHE["nc"]

    in_maps = _host_inputs(x, W_Q, W_K, W_V)
    res = run_bass_kernel_spmd(nc, in_maps, list(range(NCORES)))

    out = np.empty((B, S, E), np.float32)
    for c in range(NCORES):
        b, par = c // 2, c % 2
        o = res.results[c]["out"]  # [1024, 1024]
        for j, ch in enumerate(_CHUNKS[par]):
            out[b, ch * Q : (ch + 1) * Q] = o[j * Q : (j + 1) * Q]
    return out
